# revision 29
# baseline (speedup 1.0000x reference)
"""AMemNet (conv -> attention-LSTM) Trainium2 kernel, 8-core data parallel.

Shapes: B=64, D=2048, spatial 14x14=196, AV=512, H=1024, RG=512, STEPS=8.
Sharding: batch 64 -> 8 per core; all weights replicated.
"""

import os
import sys

import numpy as np

sys.path.insert(0, "/opt/trn_rl_repo")

import ml_dtypes  # noqa: E402
from contextlib import ExitStack  # noqa: E402

import concourse.bass as bass  # noqa: E402
import concourse.bacc as bacc  # noqa: E402
import concourse.tile as tile  # noqa: E402
from concourse import mybir  # noqa: E402
from concourse.bass_utils import run_bass_kernel_spmd  # noqa: E402

B, D, AV, AN, H, RG, STEPS = 64, 2048, 512, 196, 1024, 512, 8
NC = 8          # cores
BL = B // NC    # local batch = 8
NF = BL * AN    # 1568 free columns (batch-major x spatial)
KD = D // 128   # 16 k-chunks for conv
OC = AV // 128  # 4 output chunks (v-chunks)
HC = H // 128   # 8 hidden chunks
KC12 = (AV + H) // 128  # 12 k-chunks for gates
NK = 4          # free-dim split for psum: 4 x 392
NW = NF // NK   # 392

F32 = mybir.dt.float32
BF16 = mybir.dt.bfloat16
AF = mybir.ActivationFunctionType
OP = mybir.AluOpType

LAST_RESULT = None
_CACHE = {}


def _bf(x):
    return np.ascontiguousarray(x.astype(ml_dtypes.bfloat16))


def _f32(x):
    return np.ascontiguousarray(x.astype(np.float32))


def build_nc(W0):
    nc = bacc.Bacc()

    # ---------------- parameters ----------------
    x_p = nc.declare_dram_parameter("x", [D, NF], BF16, isOutput=False)
    convT_p = nc.declare_dram_parameter("convT", [D, AV], BF16, isOutput=False)
    cb_p = nc.declare_dram_parameter("cb", [128, OC], F32, isOutput=False)
    e1T_p = nc.declare_dram_parameter("e1T", [AV, AV], BF16, isOutput=False)
    hs1T_p = nc.declare_dram_parameter("hs1T", [AV, H], BF16, isOutput=False)
    hc1T_p = nc.declare_dram_parameter("hc1T", [AV, H], BF16, isOutput=False)
    hsb_p = nc.declare_dram_parameter("hsb", [128, HC], F32, isOutput=False)
    hcb_p = nc.declare_dram_parameter("hcb", [128, HC], F32, isOutput=False)
    eh1T_p = nc.declare_dram_parameter("eh1T", [H, AN], BF16, isOutput=False)
    eh1b_p = nc.declare_dram_parameter("eh1b", [1, AN], BF16, isOutput=False)
    eh3_p = nc.declare_dram_parameter("eh3", [128, OC], BF16, isOutput=False)
    wcatT_p = nc.declare_dram_parameter("wcatT", [AV + H, 4 * H], BF16, isOutput=False)
    gb_p = nc.declare_dram_parameter("gb", [128, 4 * HC * BL], F32, isOutput=False)
    reg1T_p = nc.declare_dram_parameter("reg1T", [H, RG], BF16, isOutput=False)
    r1b_p = nc.declare_dram_parameter("r1b", [128, RG // 128], F32, isOutput=False)
    reg4_p = nc.declare_dram_parameter("reg4", [128, RG // 128], BF16, isOutput=False)
    oneh_p = nc.declare_dram_parameter("oneh", [BL, BL * 128], BF16, isOutput=False)

    outs_p = nc.declare_dram_parameter("outs", [1, STEPS * BL], F32, isOutput=True)
    alphas_p = nc.declare_dram_parameter("alphas", [BL, STEPS, AN], BF16, isOutput=True)

    with ExitStack() as ctx:
        tc = ctx.enter_context(tile.TileContext(nc))

        # ------------- persistent pools -------------
        const = ctx.enter_context(tc.tile_pool(name="const", bufs=1))
        act = ctx.enter_context(tc.tile_pool(name="act", bufs=1))
        sc = ctx.enter_context(tc.tile_pool(name="scratch", bufs=1))
        sm = ctx.enter_context(tc.tile_pool(name="small", bufs=2))

        # small constants (sync queue, cheap)
        cb_sb = const.tile([128, OC], F32)
        nc.sync.dma_start(out=cb_sb[:, :], in_=cb_p[:, :])
        eh1b_sb = const.tile([1, AN], BF16)
        nc.sync.dma_start(out=eh1b_sb[:, :], in_=eh1b_p[:, :])
        eh3_sb = const.tile([128, OC], BF16)
        nc.sync.dma_start(out=eh3_sb[:, :], in_=eh3_p[:, :])
        gb_sb = const.tile([128, 4 * HC * BL], F32)
        nc.sync.dma_start(out=gb_sb[:, :], in_=gb_p[:, :])
        r1b_sb = const.tile([128, RG // 128], F32)
        nc.sync.dma_start(out=r1b_sb[:, :], in_=r1b_p[:, :])
        reg4_sb = const.tile([128, RG // 128], BF16)
        nc.sync.dma_start(out=reg4_sb[:, :], in_=reg4_p[:, :])
        oneh_sb = const.tile([BL, BL * 128], BF16)
        nc.sync.dma_start(out=oneh_sb[:, :], in_=oneh_p[:, :])
        hsb_sb = const.tile([128, HC], F32)
        nc.sync.dma_start(out=hsb_sb[:, :], in_=hsb_p[:, :])
        hcb_sb = const.tile([128, HC], F32)
        nc.sync.dma_start(out=hcb_sb[:, :], in_=hcb_p[:, :])
        ones_sb = const.tile([1, BL], BF16)
        nc.vector.memset(ones_sb[:, :], 1.0)
        warm_sb = const.tile([128, 512], BF16)
        nc.vector.memset(warm_sb[:, :], 0.5)
        e1T_sb = const.tile([128, OC, AV], BF16)
        for k in range(OC):
            nc.sync.dma_start(out=e1T_sb[:, k, :], in_=e1T_p[k * 128:(k + 1) * 128, :])
        # loop-phase weights go on the gpsimd (SWDGE) queue so they do not
        # block the conv-phase x loads on the sync HWDGE queue
        eh1T_sb = const.tile([128, HC, AN], BF16)
        for k in range(HC):
            nc.gpsimd.dma_start(out=eh1T_sb[:, k, :], in_=eh1T_p[k * 128:(k + 1) * 128, :])
        reg1T_sb = const.tile([128, HC, RG], BF16)
        for k in range(HC):
            nc.gpsimd.dma_start(out=reg1T_sb[:, k, :], in_=reg1T_p[k * 128:(k + 1) * 128, :])

        # activations that persist across the whole kernel
        a_sb = act.tile([128, OC, NF], BF16)       # conv output (relu'd)
        e_sb = act.tile([128, OC, NF], BF16)       # e = relu(e1 @ a)
        af_bf = act.tile([128, OC * BL], BF16)     # pooled features [v, (vc,b)]
        outs_sb = act.tile([1, STEPS * BL], F32)

        # ================= stage 1+2: conv + relu + mean, e1 =================
        with ExitStack() as cctx:
            xpool = cctx.enter_context(tc.tile_pool(name="xp", bufs=1))
            cvw = cctx.enter_context(tc.tile_pool(name="cvw", bufs=1))
            cvps = cctx.enter_context(tc.tile_pool(name="cvps", bufs=2, space="PSUM"))

            xts = []
            wT_sb = cvw.tile([128, KD, AV], BF16)
            last_x_dma = None
            for k in range(KD):
                xt = xpool.tile([128, NF], BF16, tag=f"x{k}")
                last_x_dma = nc.sync.dma_start(out=xt[:, :], in_=x_p[k * 128:(k + 1) * 128, :])
                xts.append(xt)
                nc.sync.dma_start(out=wT_sb[:, k, :], in_=convT_p[k * 128:(k + 1) * 128, :])

            af_f = cvw.tile([128, OC * BL], F32)

            # warm the PE clock (HAM) while the x DMAs stream in
            warm_ps = cvps.tile([128, NK, 512], F32, tag="cvbig")
            for w in range(24):
                nc.tensor.matmul(
                    warm_ps[:, w % NK, :],
                    warm_sb[:, 0:128], warm_sb[:, :],
                    start=True, stop=True,
                )

            for oc in range(OC):
                a_ps = cvps.tile([128, NK, 512], F32, tag="cvbig")
                for k in range(KD):
                    for nw in range(NK):
                        nc.tensor.matmul(
                            a_ps[:, nw, :NW],
                            wT_sb[:, k, oc * 128:(oc + 1) * 128],
                            xts[k][:, nw * NW:(nw + 1) * NW],
                            start=(k == 0),
                            stop=(k == KD - 1),
                        )
                for nw in range(NK):
                    nc.scalar.activation(
                        a_sb[:, oc, nw * NW:(nw + 1) * NW], a_ps[:, nw, :NW],
                        AF.Relu, bias=cb_sb[:, oc:oc + 1], scale=1.0,
                    )
                for b in range(BL):
                    nc.vector.reduce_sum(
                        af_f[:, oc * BL + b:oc * BL + b + 1],
                        a_sb[:, oc, b * AN:(b + 1) * AN],
                        axis=mybir.AxisListType.X,
                    )
            nc.scalar.activation(af_bf[:, :], af_f[:, :], AF.Copy, bias=0.0, scale=1.0 / AN)

            # e = relu(e1 @ a)
            for mc in range(OC):
                e_ps = cvps.tile([128, NK, 512], F32, tag="cvbig")
                for k in range(OC):
                    for nw in range(NK):
                        nc.tensor.matmul(
                            e_ps[:, nw, :NW],
                            e1T_sb[:, k, mc * 128:(mc + 1) * 128],
                            a_sb[:, k, nw * NW:(nw + 1) * NW],
                            start=(k == 0),
                            stop=(k == OC - 1),
                        )
                for nw in range(NK):
                    nc.scalar.activation(
                        e_sb[:, mc, nw * NW:(nw + 1) * NW], e_ps[:, nw, :NW], AF.Relu,
                    )

        # ============ loop-phase pools (conv psum/sbuf freed above) ============
        ps = ctx.enter_context(tc.tile_pool(name="ps", bufs=1, space="PSUM"))
        wcp = ctx.enter_context(tc.tile_pool(name="wcp", bufs=1))
        wcat_sb = wcp.tile([128, KC12, 4 * H], BF16)
        from bass_rust import add_dep_helper
        for k in range(KC12):
            wdma = nc.gpsimd.dma_start(out=wcat_sb[:, k, :], in_=wcatT_p[k * 128:(k + 1) * 128, :])
            if k == 0 and last_x_dma is not None:
                add_dep_helper(wdma.ins, last_x_dma.ins,
                               reason="hold wcat until x loads finish")

        # ============ stage 3: h0 = tanh(hs1@af), c0 = tanh(hc1@af) ========
        h_bf = sm.tile([128, HC * BL], BF16, tag="h")
        c_f = sm.tile([128, HC * BL], F32, tag="c")
        for src_p, bias_sb, dst, dt in (
            (hs1T_p, hsb_sb, h_bf, AF.Tanh),
            (hc1T_p, hcb_sb, c_f, AF.Tanh),
        ):
            winit = sc.tile([128, OC, H], BF16, tag="winit")
            for k in range(OC):
                nc.sync.dma_start(out=winit[:, k, :], in_=src_p[k * 128:(k + 1) * 128, :])
            for mc in range(HC):
                h_ps = ps.tile([128, BL], F32, tag="sm8", bufs=3)
                for k in range(OC):
                    nc.tensor.matmul(
                        h_ps[:, :],
                        winit[:, k, mc * 128:(mc + 1) * 128],
                        af_bf[:, k * BL:(k + 1) * BL],
                        start=(k == 0),
                        stop=(k == OC - 1),
                    )
                nc.scalar.activation(
                    dst[:, mc * BL:(mc + 1) * BL], h_ps[:, :],
                    dt, bias=bias_sb[:, mc:mc + 1], scale=1.0,
                )

        # ================= stage 4: recurrent attention loop =================
        prev_h = None

        def emit_head(tt, h_src):
            # out_tt = relu(reg1 @ h + b) . reg4
            r_bf = sm.tile([128, (RG // 128) * BL], BF16, tag="rbf", bufs=1)
            for mc in range(RG // 128):
                r_ps = ps.tile([128, BL], F32, tag="sm8", bufs=3)
                for k in range(HC):
                    nc.tensor.matmul(
                        r_ps[:, :],
                        reg1T_sb[:, k, mc * 128:(mc + 1) * 128],
                        h_src[:, k * BL:(k + 1) * BL],
                        start=(k == 0),
                        stop=(k == HC - 1),
                    )
                nc.scalar.activation(
                    r_bf[:, mc * BL:(mc + 1) * BL], r_ps[:, :],
                    AF.Relu, bias=r1b_sb[:, mc:mc + 1], scale=1.0,
                )
            o_ps = ps.tile([1, BL], F32, tag="sm8", bufs=3)
            for mc in range(RG // 128):
                nc.tensor.matmul(
                    o_ps[:, :],
                    reg4_sb[:, mc:mc + 1],
                    r_bf[:, mc * BL:(mc + 1) * BL],
                    start=(mc == 0),
                    stop=(mc == RG // 128 - 1),
                )
            nc.scalar.activation(outs_sb[:, tt * BL:(tt + 1) * BL], o_ps[:, :], AF.Copy)

        # Gate order in wcat/gb is host-reordered to [i, f, o, g] so one
        # fused Sigmoid covers gall[:, 0:3, :].
        # relu(e+eh) = max(e, -eh) + eh: the +eh term is constant across v,
        # so logits = sum_v w_v * max(e, -eh)  +  W0 * eh  (W0 = sum_v w_v).
        def gh_slice(g_lo, g_hi, gh_ps, h_src):
            for g in range(g_lo, g_hi):
                for hcj in range(HC):
                    m0 = g * H + hcj * 128
                    for k in range(OC, KC12):
                        nc.tensor.matmul(
                            gh_ps[:, g, hcj * BL:(hcj + 1) * BL],
                            wcat_sb[:, k, m0:m0 + 128],
                            h_src[:, (k - OC) * BL:(k - OC + 1) * BL],
                            start=(k == OC),
                            stop=(k == KC12 - 1),
                        )

        for t in range(STEPS):
            # --- eh[b, n] = (h @ eh1_w.T + eh1_b), directly as [BL, AN]
            eh_ps = ps.tile([BL, 256], F32, tag="sm8", bufs=3)
            for k in range(HC):
                nc.tensor.matmul(
                    eh_ps[:, :AN],
                    h_bf[:, k * BL:(k + 1) * BL],
                    eh1T_sb[:, k, :],
                    start=(k == 0),
                    stop=False,
                )
            nc.tensor.matmul(
                eh_ps[:, :AN], ones_sb[:, :], eh1b_sb[:, :], start=False, stop=True,
            )
            nehT_sb = sm.tile([BL, AN], BF16, tag="ehT", bufs=1)   # -eh
            nc.scalar.activation(nehT_sb[:, :], eh_ps[:, :AN], AF.Copy, bias=0.0, scale=-1.0)

            if prev_h is not None:
                emit_head(t - 1, prev_h)

            # --- broadcast -eh across partitions -> SBUF bf16
            nehb_sb = sc.tile([128, BL, AN], BF16, tag="bcast", bufs=1)
            for j in range(4):
                bc_ps = ps.tile([128, 2, AN], F32, tag=f"bc{j % 2}", bufs=1)
                for i in range(2):
                    b = 2 * j + i
                    nc.tensor.matmul(
                        bc_ps[:, i, :],
                        oneh_sb[:, b * 128:(b + 1) * 128],
                        nehT_sb[:, :],
                        start=True,
                        stop=True,
                    )
                nc.scalar.activation(nehb_sb[:, 2 * j:2 * j + 2, :], bc_ps[:, :, :], AF.Copy)

            gh_ps = ps.tile([128, 4, HC * BL], F32, tag="gatesh", bufs=1)

            # --- m = max(e, -ehb)
            s_sb = sc.tile([128, OC, NF], BF16, tag="s", bufs=1)
            for vc in range(OC):
                nc.vector.tensor_tensor(
                    s_sb[:, vc, :], e_sb[:, vc, :],
                    nehb_sb[:, :, :].rearrange("p b n -> p (b n)"),
                    op=OP.max,
                )

            # --- logits_m = sum_v eh3_w[v] * m[v, :]
            lg_sb = sm.tile([1, NF], F32, tag="lg", bufs=1)
            for nk in range(NK):
                lg_ps = ps.tile([1, NW], F32, tag="sm8", bufs=3)
                for vc in range(OC):
                    nc.tensor.matmul(
                        lg_ps[:, :],
                        eh3_sb[:, vc:vc + 1],
                        s_sb[:, vc, nk * NW:(nk + 1) * NW],
                        start=(vc == 0),
                        stop=(vc == OC - 1),
                    )
                nc.scalar.activation(lg_sb[:, nk * NW:(nk + 1) * NW], lg_ps[:, :], AF.Copy)

            # --- gates h-part slice A (dense PE block; hides softmax chain)
            gh_slice(0, 2, gh_ps, h_bf)

            # --- reshape logits, add back W0*eh, softmax over n
            lgT = sm.tile([BL, AN], F32, tag="lgT", bufs=1)
            nc.sync.dma_start(out=lgT[:, :], in_=lg_sb[:, :])
            nc.vector.scalar_tensor_tensor(
                lgT[:, :], nehT_sb[:, :], -W0, lgT[:, :],
                op0=OP.mult, op1=OP.add,
            )
            exp_sb = sm.tile([BL, AN], F32, tag="exp", bufs=1)
            sumx = sm.tile([BL, 1], F32, tag="sumx", bufs=1)
            nc.scalar.activation(exp_sb[:, :], lgT[:, :], AF.Exp, accum_out=sumx[:, :])
            rcp = sm.tile([BL, 1], F32, tag="rcp", bufs=1)
            nc.vector.reciprocal(rcp[:, :], sumx[:, :])
            alpha_bf = sm.tile([BL, AN], BF16, tag="alphab", bufs=1)
            nc.vector.tensor_scalar_mul(alpha_bf[:, :], exp_sb[:, :], rcp[:, :])
            nc.sync.dma_start(out=alphas_p[:, t, :], in_=alpha_bf[:, :])

            # --- broadcast alpha -> SBUF bf16
            alb_sb = sc.tile([128, BL, AN], BF16, tag="bcast", bufs=1)
            for j in range(4):
                bc_ps = ps.tile([128, 2, AN], F32, tag=f"bc{j % 2}", bufs=1)
                for i in range(2):
                    b = 2 * j + i
                    nc.tensor.matmul(
                        bc_ps[:, i, :],
                        oneh_sb[:, b * 128:(b + 1) * 128],
                        alpha_bf[:, :],
                        start=True,
                        stop=True,
                    )
                nc.scalar.activation(alb_sb[:, 2 * j:2 * j + 2, :], bc_ps[:, :, :], AF.Copy)

            # --- gates h-part slice B (hides z mul/reduce)
            gh_slice(2, 4, gh_ps, h_bf)

            # --- z = sum_n a*alb, then gates z-part (window-contiguous groups:
            #     start=True clears has_written for the whole bank, so a
            #     window's accumulation may not interleave with another's)
            g_ps0 = ps.tile([128, 4, HC * BL // 2], F32, tag="gates0", bufs=1)
            g_ps1 = ps.tile([128, 4, HC * BL // 2], F32, tag="gates1", bufs=1)
            z_f = sm.tile([128, OC * BL], F32, tag="zf", bufs=1)
            z_bf = sm.tile([128, OC * BL], BF16, tag="zbf", bufs=1)
            hs_sb = sm.tile([128, BL * (AN // 2)], BF16, tag="hsum", bufs=1)
            for vc in range(OC):
                nc.vector.tensor_tensor(
                    s_sb[:, vc, :], a_sb[:, vc, :],
                    alb_sb[:, :, :].rearrange("p b n -> p (b n)"),
                    op=OP.mult,
                )
                pv = s_sb[:, vc, :].rearrange("p (b h n) -> p b h n", b=BL, h=2)
                nc.vector.tensor_tensor(
                    hs_sb[:, :].rearrange("p (b n) -> p b n", b=BL),
                    pv[:, :, 0, :], pv[:, :, 1, :], op=OP.add,
                )
                nc.vector.reduce_sum(
                    z_f[:, vc * BL:(vc + 1) * BL],
                    hs_sb[:, :].rearrange("p (b n) -> p b n", b=BL),
                    axis=mybir.AxisListType.X,
                )
                nc.vector.tensor_copy(
                    z_bf[:, vc * BL:(vc + 1) * BL], z_f[:, vc * BL:(vc + 1) * BL])
            c_prev = c_f
            gall = sm.tile([128, 4, HC * BL], F32, tag="gall", bufs=1)
            sig = sm.tile([128, 3, HC * BL], F32, tag="sig", bufs=1)
            tg = sm.tile([128, HC * BL], F32, tag="tg", bufs=1)
            t1 = sm.tile([128, HC * BL], F32, tag="t1", bufs=1)
            t2 = sm.tile([128, HC * BL], F32, tag="t2", bufs=1)
            c_f = sm.tile([128, HC * BL], F32, tag="c")
            tc_f = sm.tile([128, HC * BL], F32, tag="tc", bufs=1)
            h_bf = sm.tile([128, HC * BL], BF16, tag="h")
            HB = HC * BL // 2
            for half in range(2):
                g_ps = (g_ps0, g_ps1)[half]
                for g in range(4):
                    for hcj in range(half * HC // 2, (half + 1) * HC // 2):
                        m0 = g * H + hcj * 128
                        co = (hcj - half * HC // 2) * BL
                        for vc in range(OC):
                            nc.tensor.matmul(
                                g_ps[:, g, co:co + BL],
                                wcat_sb[:, vc, m0:m0 + 128],
                                z_bf[:, vc * BL:(vc + 1) * BL],
                                start=(vc == 0),
                                stop=(vc == OC - 1),
                            )
                # --- LSTM cell on this hidden half; gate order [i, f, o, g]
                hsl = slice(half * HB, (half + 1) * HB)
                nc.vector.tensor_tensor(
                    gall[:, :, hsl], gh_ps[:, :, hsl],
                    gb_sb[:, :].rearrange("p (g c) -> p g c", g=4)[:, :, hsl],
                    op=OP.add,
                )
                nc.vector.tensor_tensor(
                    gall[:, :, hsl], g_ps[:, :, :], gall[:, :, hsl], op=OP.add,
                )
                nc.scalar.activation(sig[:, :, hsl], gall[:, 0:3, hsl], AF.Sigmoid)
                nc.scalar.activation(tg[:, hsl], gall[:, 3, hsl], AF.Tanh)
                nc.vector.tensor_mul(t1[:, hsl], sig[:, 0, hsl], tg[:, hsl])
                nc.vector.tensor_mul(t2[:, hsl], sig[:, 1, hsl], c_prev[:, hsl])
                nc.vector.tensor_add(c_f[:, hsl], t1[:, hsl], t2[:, hsl])
                nc.scalar.activation(tc_f[:, hsl], c_f[:, hsl], AF.Tanh)
                nc.vector.tensor_mul(h_bf[:, hsl], sig[:, 2, hsl], tc_f[:, hsl])

            prev_h = h_bf

        emit_head(STEPS - 1, h_bf)
        nc.sync.dma_start(out=outs_p[:, :], in_=outs_sb[:, :])

    nc.compile()
    return nc


def _prep_inputs(inputs):
    ins = {k: np.asarray(v, np.float32) for k, v in inputs.items()}
    eps = 1e-5
    scale = ins["bn_gamma"] / np.sqrt(ins["bn_var"] + eps)
    wc = ins["inconv_w"] * scale[:, None]                     # [512, 2048]
    bc = ins["inconv_b"] * scale + ins["bn_beta"] - ins["bn_mean"] * scale

    def pack_cols(v, ncol):  # [ncol*128] -> [128, ncol]
        return _f32(v.reshape(ncol, 128).T)

    def reord(w):  # [4096, ...] pytorch i,f,g,o -> i,f,o,g
        return np.concatenate([w[:H], w[H:2 * H], w[3 * H:], w[2 * H:3 * H]], axis=0)

    bb = reord(ins["b_ih"] + ins["b_hh"])                     # [4096]
    # gb[p, g*HC*BL + c*BL + b] = bb[g*1024 + c*128 + p]
    gb = np.repeat(bb.reshape(4, HC, 128).transpose(2, 0, 1).reshape(128, 4 * HC),
                   BL, axis=1)

    oneh = np.zeros((BL, BL * 128), np.float32)
    for b in range(BL):
        oneh[b, b * 128:(b + 1) * 128] = 1.0

    shared = {
        "convT": _bf(wc.T),
        "cb": pack_cols(bc, OC),
        "e1T": _bf(ins["e1_w"].T),
        "hs1T": _bf(ins["hs1_w"].T),
        "hc1T": _bf(ins["hc1_w"].T),
        "hsb": pack_cols(ins["hs1_b"], HC),
        "hcb": pack_cols(ins["hc1_b"], HC),
        "eh1T": _bf(ins["eh1_w"].T),
        "eh1b": _bf(ins["eh1_b"][None, :]),
        "eh3": _bf(ins["eh3_w"].reshape(OC, 128).T),
        "wcatT": _bf(reord(np.concatenate([ins["w_ih"], ins["w_hh"]], axis=1)).T),
        "gb": _f32(gb),
        "reg1T": _bf(ins["reg1_w"].T),
        "r1b": pack_cols(ins["reg1_b"], RG // 128),
        "reg4": _bf(ins["reg4_w"].reshape(RG // 128, 128).T),
        "oneh": _bf(oneh),
    }

    x = ins["x"].reshape(B, D, AN)
    in_maps = []
    for c in range(NC):
        xs = x[c * BL:(c + 1) * BL]                 # [8, 2048, 196]
        xs = xs.transpose(1, 0, 2).reshape(D, NF)   # [2048, 1568]
        m = dict(shared)
        m["x"] = _bf(xs)
        in_maps.append(m)
    return in_maps, float(ins["reg4_b"])


def _install_trace_hook():
    """The agent image's antenv lacks axon_hooks; synthesize it so
    run_bass_kernel_spmd(trace=True) can reach NTFF profiling."""
    import types

    try:
        from antenv.axon_hooks import get_axon_ntff_profile_hook  # noqa: F401
        return
    except ImportError:
        pass
    import antenv

    mod = types.ModuleType("antenv.axon_hooks")
    _h = [None]
    mod.set_axon_ntff_profile_hook = lambda h: _h.__setitem__(0, h)
    mod.get_axon_ntff_profile_hook = lambda: _h[0]
    sys.modules["antenv.axon_hooks"] = mod
    antenv.axon_hooks = mod
    sys.path.insert(0, "/root/.axon_site")
    from trn_agent_boot.trn_boot import _ntff_profile_via_ctypes

    hook = _ntff_profile_via_ctypes("/opt/axon/libaxon_pjrt.so")
    if hook is not None:
        mod.set_axon_ntff_profile_hook(hook)


def kernel(**inputs):
    global LAST_RESULT
    if "nc" not in _CACHE:
        _CACHE["W0"] = float(
            np.asarray(inputs["eh3_w"], np.float32)
            .astype(ml_dtypes.bfloat16).astype(np.float32).sum())
        _CACHE["nc"] = build_nc(_CACHE["W0"])
    if bool(int(os.environ.get("KERNEL_TRACE", "0"))):
        _install_trace_hook()
    nc = _CACHE["nc"]
    in_maps, reg4_b = _prep_inputs(inputs)
    res = run_bass_kernel_spmd(
        nc, in_maps, core_ids=list(range(NC)),
        trace=bool(int(os.environ.get("KERNEL_TRACE", "0"))),
    )
    LAST_RESULT = res
    out_seq = np.zeros((B, STEPS), np.float32)
    alphas = np.zeros((B, STEPS, AN), np.float32)
    for c in range(NC):
        r = res.results[c]
        out_seq[c * BL:(c + 1) * BL] = r["outs"].reshape(STEPS, BL).T + reg4_b
        alphas[c * BL:(c + 1) * BL] = np.asarray(r["alphas"], np.float32)
    return out_seq, alphas


# revision 31
# speedup vs baseline: 1.1757x; 1.1757x over previous
"""AMemNet (conv -> attention-LSTM) Trainium2 kernel, 8-core data parallel.

Shapes: B=64, D=2048, spatial 14x14=196, AV=512, H=1024, RG=512, STEPS=8.
Sharding: batch 64 -> 8 per core; all weights replicated.
"""

import os
import sys

import numpy as np

sys.path.insert(0, "/opt/trn_rl_repo")

import ml_dtypes  # noqa: E402
from contextlib import ExitStack  # noqa: E402

import concourse.bass as bass  # noqa: E402
import concourse.bacc as bacc  # noqa: E402
import concourse.tile as tile  # noqa: E402
from concourse import mybir  # noqa: E402
from concourse.bass_utils import run_bass_kernel_spmd  # noqa: E402

B, D, AV, AN, H, RG, STEPS = 64, 2048, 512, 196, 1024, 512, 8
NC = 8          # cores
BL = B // NC    # local batch = 8
NF = BL * AN    # 1568 free columns (batch-major x spatial)
KD = D // 128   # 16 k-chunks for conv
OC = AV // 128  # 4 output chunks (v-chunks)
HC = H // 128   # 8 hidden chunks
KC12 = (AV + H) // 128  # 12 k-chunks for gates
NK = 4          # free-dim split for psum: 4 x 392
NW = NF // NK   # 392

F32 = mybir.dt.float32
BF16 = mybir.dt.bfloat16
AF = mybir.ActivationFunctionType
OP = mybir.AluOpType

LAST_RESULT = None
_CACHE = {}


def _bf(x):
    return np.ascontiguousarray(x.astype(ml_dtypes.bfloat16))


def _f32(x):
    return np.ascontiguousarray(x.astype(np.float32))


def build_nc(W0):
    nc = bacc.Bacc()

    # ---------------- parameters ----------------
    x_p = nc.declare_dram_parameter("x", [D, NF], BF16, isOutput=False)
    convT_p = nc.declare_dram_parameter("convT", [D, AV], BF16, isOutput=False)
    cb_p = nc.declare_dram_parameter("cb", [128, OC], F32, isOutput=False)
    e1T_p = nc.declare_dram_parameter("e1T", [AV, AV], BF16, isOutput=False)
    hs1T_p = nc.declare_dram_parameter("hs1T", [AV, H], BF16, isOutput=False)
    hc1T_p = nc.declare_dram_parameter("hc1T", [AV, H], BF16, isOutput=False)
    hsb_p = nc.declare_dram_parameter("hsb", [128, HC], F32, isOutput=False)
    hcb_p = nc.declare_dram_parameter("hcb", [128, HC], F32, isOutput=False)
    eh1T_p = nc.declare_dram_parameter("eh1T", [H, AN], BF16, isOutput=False)
    eh1b_p = nc.declare_dram_parameter("eh1b", [1, AN], BF16, isOutput=False)
    eh3_p = nc.declare_dram_parameter("eh3", [128, OC], BF16, isOutput=False)
    wcatT_p = nc.declare_dram_parameter("wcatT", [AV + H, 4 * H], BF16, isOutput=False)
    gb_p = nc.declare_dram_parameter("gb", [128, 4 * HC * BL], F32, isOutput=False)
    reg1T_p = nc.declare_dram_parameter("reg1T", [H, RG], BF16, isOutput=False)
    r1b_p = nc.declare_dram_parameter("r1b", [128, RG // 128], F32, isOutput=False)
    reg4_p = nc.declare_dram_parameter("reg4", [128, RG // 128], BF16, isOutput=False)
    oneh_p = nc.declare_dram_parameter("oneh", [BL, BL * 128], BF16, isOutput=False)

    outs_p = nc.declare_dram_parameter("outs", [1, STEPS * BL], F32, isOutput=True)
    alphas_p = nc.declare_dram_parameter("alphas", [BL, STEPS, AN], BF16, isOutput=True)

    with ExitStack() as ctx:
        tc = ctx.enter_context(tile.TileContext(nc))

        # ------------- persistent pools -------------
        const = ctx.enter_context(tc.tile_pool(name="const", bufs=1))
        act = ctx.enter_context(tc.tile_pool(name="act", bufs=1))
        sc = ctx.enter_context(tc.tile_pool(name="scratch", bufs=1))
        sm = ctx.enter_context(tc.tile_pool(name="small", bufs=2))

        # small constants (sync queue, cheap)
        cb_sb = const.tile([128, OC], F32)
        nc.sync.dma_start(out=cb_sb[:, :], in_=cb_p[:, :])
        eh1b_sb = const.tile([1, AN], BF16)
        nc.sync.dma_start(out=eh1b_sb[:, :], in_=eh1b_p[:, :])
        eh3_sb = const.tile([128, OC], BF16)
        nc.sync.dma_start(out=eh3_sb[:, :], in_=eh3_p[:, :])
        gb_sb = const.tile([128, 4 * HC * BL], F32)
        nc.sync.dma_start(out=gb_sb[:, :], in_=gb_p[:, :])
        r1b_sb = const.tile([128, RG // 128], F32)
        nc.sync.dma_start(out=r1b_sb[:, :], in_=r1b_p[:, :])
        reg4_sb = const.tile([128, RG // 128], BF16)
        nc.sync.dma_start(out=reg4_sb[:, :], in_=reg4_p[:, :])
        oneh_sb = const.tile([BL, BL * 128], BF16)
        nc.sync.dma_start(out=oneh_sb[:, :], in_=oneh_p[:, :])
        hsb_sb = const.tile([128, HC], F32)
        nc.sync.dma_start(out=hsb_sb[:, :], in_=hsb_p[:, :])
        hcb_sb = const.tile([128, HC], F32)
        nc.sync.dma_start(out=hcb_sb[:, :], in_=hcb_p[:, :])
        ones_sb = const.tile([1, BL], BF16)
        nc.vector.memset(ones_sb[:, :], 1.0)
        warm_sb = const.tile([128, 512], BF16)
        nc.vector.memset(warm_sb[:, :], 0.5)
        e1T_sb = const.tile([128, OC, AV], BF16)
        for k in range(OC):
            nc.sync.dma_start(out=e1T_sb[:, k, :], in_=e1T_p[k * 128:(k + 1) * 128, :])
        # loop-phase weights go on the gpsimd (SWDGE) queue so they do not
        # block the conv-phase x loads on the sync HWDGE queue
        eh1T_sb = const.tile([128, HC, AN], BF16)
        for k in range(HC):
            nc.gpsimd.dma_start(out=eh1T_sb[:, k, :], in_=eh1T_p[k * 128:(k + 1) * 128, :])
        reg1T_sb = const.tile([128, HC, RG], BF16)
        for k in range(HC):
            nc.gpsimd.dma_start(out=reg1T_sb[:, k, :], in_=reg1T_p[k * 128:(k + 1) * 128, :])

        # activations that persist across the whole kernel
        a_sb = act.tile([128, OC, NF], BF16)       # conv output (relu'd)
        e_sb = act.tile([128, OC, NF], BF16)       # e = relu(e1 @ a)
        af_bf = act.tile([128, OC * BL], BF16)     # pooled features [v, (vc,b)]
        outs_sb = act.tile([1, STEPS * BL], F32)

        # ================= stage 1+2: conv + relu + mean, e1 =================
        with ExitStack() as cctx:
            xpool = cctx.enter_context(tc.tile_pool(name="xp", bufs=1))
            cvw = cctx.enter_context(tc.tile_pool(name="cvw", bufs=1))
            cvps = cctx.enter_context(tc.tile_pool(name="cvps", bufs=2, space="PSUM"))

            xts = []
            wT_sb = cvw.tile([128, KD, AV], BF16)
            last_x_dma = None
            for k in range(KD):
                xt = xpool.tile([128, NF], BF16, tag=f"x{k}")
                last_x_dma = nc.sync.dma_start(out=xt[:, :], in_=x_p[k * 128:(k + 1) * 128, :])
                xts.append(xt)
                nc.sync.dma_start(out=wT_sb[:, k, :], in_=convT_p[k * 128:(k + 1) * 128, :])

            af_f = cvw.tile([128, OC * BL], F32)

            # warm the PE clock (HAM) while the x DMAs stream in
            warm_ps = cvps.tile([128, NK, 512], F32, tag="cvbig")
            for w in range(24):
                nc.tensor.matmul(
                    warm_ps[:, w % NK, :],
                    warm_sb[:, 0:128], warm_sb[:, :],
                    start=True, stop=True,
                )

            for oc in range(OC):
                a_ps = cvps.tile([128, NK, 512], F32, tag="cvbig")
                for k in range(KD):
                    for nw in range(NK):
                        nc.tensor.matmul(
                            a_ps[:, nw, :NW],
                            wT_sb[:, k, oc * 128:(oc + 1) * 128],
                            xts[k][:, nw * NW:(nw + 1) * NW],
                            start=(k == 0),
                            stop=(k == KD - 1),
                        )
                for nw in range(NK):
                    nc.scalar.activation(
                        a_sb[:, oc, nw * NW:(nw + 1) * NW], a_ps[:, nw, :NW],
                        AF.Relu, bias=cb_sb[:, oc:oc + 1], scale=1.0,
                    )
                for b in range(BL):
                    nc.vector.reduce_sum(
                        af_f[:, oc * BL + b:oc * BL + b + 1],
                        a_sb[:, oc, b * AN:(b + 1) * AN],
                        axis=mybir.AxisListType.X,
                    )
            nc.scalar.activation(af_bf[:, :], af_f[:, :], AF.Copy, bias=0.0, scale=1.0 / AN)

            # e = relu(e1 @ a)
            for mc in range(OC):
                e_ps = cvps.tile([128, NK, 512], F32, tag="cvbig")
                for k in range(OC):
                    for nw in range(NK):
                        nc.tensor.matmul(
                            e_ps[:, nw, :NW],
                            e1T_sb[:, k, mc * 128:(mc + 1) * 128],
                            a_sb[:, k, nw * NW:(nw + 1) * NW],
                            start=(k == 0),
                            stop=(k == OC - 1),
                        )
                for nw in range(NK):
                    nc.scalar.activation(
                        e_sb[:, mc, nw * NW:(nw + 1) * NW], e_ps[:, nw, :NW], AF.Relu,
                    )

        # ============ loop-phase pools (conv psum/sbuf freed above) ============
        ps = ctx.enter_context(tc.tile_pool(name="ps", bufs=1, space="PSUM"))
        wcp = ctx.enter_context(tc.tile_pool(name="wcp", bufs=1))
        wcat_sb = wcp.tile([128, KC12, 4 * H], BF16)
        from bass_rust import add_dep_helper
        for k in range(KC12):
            wdma = nc.gpsimd.dma_start(out=wcat_sb[:, k, :], in_=wcatT_p[k * 128:(k + 1) * 128, :])
            if k == 0 and last_x_dma is not None:
                add_dep_helper(wdma.ins, last_x_dma.ins,
                               reason="hold wcat until x loads finish")

        # ============ stage 3: h0 = tanh(hs1@af), c0 = tanh(hc1@af) ========
        h_bf = sm.tile([128, HC * BL], BF16, tag="h")
        c_f = sm.tile([128, HC * BL], F32, tag="c")
        for src_p, bias_sb, dst, dt in (
            (hs1T_p, hsb_sb, h_bf, AF.Tanh),
            (hc1T_p, hcb_sb, c_f, AF.Tanh),
        ):
            winit = sc.tile([128, OC, H], BF16, tag="winit")
            for k in range(OC):
                nc.sync.dma_start(out=winit[:, k, :], in_=src_p[k * 128:(k + 1) * 128, :])
            for mc in range(HC):
                h_ps = ps.tile([128, BL], F32, tag="sm8", bufs=3)
                for k in range(OC):
                    nc.tensor.matmul(
                        h_ps[:, :],
                        winit[:, k, mc * 128:(mc + 1) * 128],
                        af_bf[:, k * BL:(k + 1) * BL],
                        start=(k == 0),
                        stop=(k == OC - 1),
                    )
                nc.scalar.activation(
                    dst[:, mc * BL:(mc + 1) * BL], h_ps[:, :],
                    dt, bias=bias_sb[:, mc:mc + 1], scale=1.0,
                )
            if dst is c_f:
                nc.vector.tensor_scalar_mul(c_f[:, :], c_f[:, :], 2.0)

        # ================= stage 4: recurrent attention loop =================
        prev_h = None

        def emit_head(tt, h_src):
            # out_tt = relu(reg1 @ h + b) . reg4
            r_bf = sm.tile([128, (RG // 128) * BL], BF16, tag="rbf", bufs=1)
            for mc in range(RG // 128):
                r_ps = ps.tile([128, BL], F32, tag="sm8", bufs=3)
                for k in range(HC):
                    nc.tensor.matmul(
                        r_ps[:, :],
                        reg1T_sb[:, k, mc * 128:(mc + 1) * 128],
                        h_src[:, k * BL:(k + 1) * BL],
                        start=(k == 0),
                        stop=(k == HC - 1),
                    )
                nc.scalar.activation(
                    r_bf[:, mc * BL:(mc + 1) * BL], r_ps[:, :],
                    AF.Relu, bias=r1b_sb[:, mc:mc + 1], scale=1.0,
                )
            o_ps = ps.tile([1, BL], F32, tag="sm8", bufs=3)
            for mc in range(RG // 128):
                nc.tensor.matmul(
                    o_ps[:, :],
                    reg4_sb[:, mc:mc + 1],
                    r_bf[:, mc * BL:(mc + 1) * BL],
                    start=(mc == 0),
                    stop=(mc == RG // 128 - 1),
                )
            nc.scalar.activation(outs_sb[:, tt * BL:(tt + 1) * BL], o_ps[:, :], AF.Copy)

        # Gate order in wcat/gb is host-reordered to [i, f, o, g] so one
        # fused Sigmoid covers gall[:, 0:3, :].
        # relu(e+eh) = max(e, -eh) + eh: the +eh term is constant across v,
        # so logits = sum_v w_v * max(e, -eh)  +  W0 * eh  (W0 = sum_v w_v).
        def gh_slice(g_lo, g_hi, gh_ps, h_src):
            for g in range(g_lo, g_hi):
                for hcj in range(HC):
                    m0 = g * H + hcj * 128
                    for k in range(OC, KC12):
                        nc.tensor.matmul(
                            gh_ps[:, g, hcj * BL:(hcj + 1) * BL],
                            wcat_sb[:, k, m0:m0 + 128],
                            h_src[:, (k - OC) * BL:(k - OC + 1) * BL],
                            start=(k == OC),
                            stop=(k == KC12 - 1),
                        )

        for t in range(STEPS):
            # --- eh[b, n] = (h @ eh1_w.T + eh1_b), directly as [BL, AN]
            eh_ps = ps.tile([BL, 256], F32, tag="sm8", bufs=3)
            for k in range(HC):
                nc.tensor.matmul(
                    eh_ps[:, :AN],
                    h_bf[:, k * BL:(k + 1) * BL],
                    eh1T_sb[:, k, :],
                    start=(k == 0),
                    stop=False,
                )
            nc.tensor.matmul(
                eh_ps[:, :AN], ones_sb[:, :], eh1b_sb[:, :], start=False, stop=True,
            )
            nehT_sb = sm.tile([BL, AN], BF16, tag="ehT", bufs=1)   # -eh
            nc.scalar.activation(nehT_sb[:, :], eh_ps[:, :AN], AF.Copy, bias=0.0, scale=-1.0)

            if prev_h is not None:
                emit_head(t - 1, prev_h)

            # --- broadcast -eh across partitions -> SBUF bf16
            nehb_sb = sc.tile([128, BL, AN], BF16, tag="bcast", bufs=1)
            for j in range(4):
                bc_ps = ps.tile([128, 2, AN], F32, tag=f"bc{j % 2}", bufs=1)
                for i in range(2):
                    b = 2 * j + i
                    nc.tensor.matmul(
                        bc_ps[:, i, :],
                        oneh_sb[:, b * 128:(b + 1) * 128],
                        nehT_sb[:, :],
                        start=True,
                        stop=True,
                    )
                nc.scalar.activation(nehb_sb[:, 2 * j:2 * j + 2, :], bc_ps[:, :, :], AF.Copy)

            gh_ps = ps.tile([128, 4, HC * BL], F32, tag="gatesh", bufs=1)

            # --- m = max(e, -ehb)
            s_sb = sc.tile([128, OC, NF], BF16, tag="s", bufs=1)
            for vc in range(OC):
                nc.vector.tensor_tensor(
                    s_sb[:, vc, :], e_sb[:, vc, :],
                    nehb_sb[:, :, :].rearrange("p b n -> p (b n)"),
                    op=OP.max,
                )

            # --- logits_m = sum_v eh3_w[v] * m[v, :]
            lg_sb = sm.tile([1, NF], F32, tag="lg", bufs=1)
            for nk in range(NK):
                lg_ps = ps.tile([1, NW], F32, tag="sm8", bufs=3)
                for vc in range(OC):
                    nc.tensor.matmul(
                        lg_ps[:, :],
                        eh3_sb[:, vc:vc + 1],
                        s_sb[:, vc, nk * NW:(nk + 1) * NW],
                        start=(vc == 0),
                        stop=(vc == OC - 1),
                    )
                nc.scalar.activation(lg_sb[:, nk * NW:(nk + 1) * NW], lg_ps[:, :], AF.Copy)

            # --- gates h-part slice A (dense PE block; hides softmax chain)
            gh_slice(0, 2, gh_ps, h_bf)

            # --- reshape logits, add back W0*eh, softmax over n
            lgT = sm.tile([BL, AN], F32, tag="lgT", bufs=1)
            nc.sync.dma_start(out=lgT[:, :], in_=lg_sb[:, :])
            nc.vector.scalar_tensor_tensor(
                lgT[:, :], nehT_sb[:, :], -W0, lgT[:, :],
                op0=OP.mult, op1=OP.add,
            )
            exp_sb = sm.tile([BL, AN], F32, tag="exp", bufs=1)
            sumx = sm.tile([BL, 1], F32, tag="sumx", bufs=1)
            nc.scalar.activation(exp_sb[:, :], lgT[:, :], AF.Exp, accum_out=sumx[:, :])
            rcp = sm.tile([BL, 1], F32, tag="rcp", bufs=1)
            nc.vector.reciprocal(rcp[:, :], sumx[:, :])
            alpha_bf = sm.tile([BL, AN], BF16, tag="alphab", bufs=1)
            nc.vector.tensor_scalar_mul(alpha_bf[:, :], exp_sb[:, :], rcp[:, :])
            nc.sync.dma_start(out=alphas_p[:, t, :], in_=alpha_bf[:, :])

            # --- broadcast alpha -> SBUF bf16
            alb_sb = sc.tile([128, BL, AN], BF16, tag="bcast", bufs=1)
            for j in range(4):
                bc_ps = ps.tile([128, 2, AN], F32, tag=f"bc{j % 2}", bufs=1)
                for i in range(2):
                    b = 2 * j + i
                    nc.tensor.matmul(
                        bc_ps[:, i, :],
                        oneh_sb[:, b * 128:(b + 1) * 128],
                        alpha_bf[:, :],
                        start=True,
                        stop=True,
                    )
                nc.scalar.activation(alb_sb[:, 2 * j:2 * j + 2, :], bc_ps[:, :, :], AF.Copy)

            # --- gates h-part slice B (hides z mul/reduce)
            gh_slice(2, 4, gh_ps, h_bf)

            # --- z = sum_n a*alb, then gates z-part (window-contiguous groups:
            #     start=True clears has_written for the whole bank, so a
            #     window's accumulation may not interleave with another's)
            g_ps0 = ps.tile([128, 4, HC * BL], F32, tag="gates0", bufs=1)
            g_ps1 = ps.tile([128, 4, HC * BL], F32, tag="gates1", bufs=1)
            z_f = sm.tile([128, OC * BL], F32, tag="zf", bufs=1)
            z_bf = sm.tile([128, OC * BL], BF16, tag="zbf", bufs=1)
            hs_sb = sm.tile([128, BL * (AN // 2)], BF16, tag="hsum", bufs=1)
            for vc in range(OC):
                nc.vector.tensor_tensor(
                    s_sb[:, vc, :], a_sb[:, vc, :],
                    alb_sb[:, :, :].rearrange("p b n -> p (b n)"),
                    op=OP.mult,
                )
                pv = s_sb[:, vc, :].rearrange("p (b h n) -> p b h n", b=BL, h=2)
                nc.vector.tensor_tensor(
                    hs_sb[:, :].rearrange("p (b n) -> p b n", b=BL),
                    pv[:, :, 0, :], pv[:, :, 1, :], op=OP.add,
                )
                nc.vector.reduce_sum(
                    z_f[:, vc * BL:(vc + 1) * BL],
                    hs_sb[:, :].rearrange("p (b n) -> p b n", b=BL),
                    axis=mybir.AxisListType.X,
                )
                nc.vector.tensor_copy(
                    z_bf[:, vc * BL:(vc + 1) * BL], z_f[:, vc * BL:(vc + 1) * BL])
            c_prev = c_f
            gall = sm.tile([128, 4, HC * BL], F32, tag="gall", bufs=1)
            sig = sm.tile([128, 3, HC * BL], F32, tag="sig", bufs=1)
            tg = sm.tile([128, HC * BL], F32, tag="tg", bufs=1)
            t1 = sm.tile([128, HC * BL], F32, tag="t1", bufs=1)
            t2 = sm.tile([128, HC * BL], F32, tag="t2", bufs=1)
            c_f = sm.tile([128, HC * BL], F32, tag="c")
            tc_f = sm.tile([128, HC * BL], F32, tag="tc", bufs=1)
            h_bf = sm.tile([128, HC * BL], BF16, tag="h")
            HB = HC * BL // 2
            # z-part split by vc-PAIRS into two psum tiles: the first 64
            # matmuls start once z chunks 0,1 are reduced instead of all
            # 128 waiting for the full z. Windows ordered hcj-major so the
            # cell halves unblock early.
            for pair, g_ps in ((0, g_ps0), (1, g_ps1)):
                for hcj in range(HC):
                    for g in range(4):
                        m0 = g * H + hcj * 128
                        for vc in (2 * pair, 2 * pair + 1):
                            nc.tensor.matmul(
                                g_ps[:, g, hcj * BL:(hcj + 1) * BL],
                                wcat_sb[:, vc, m0:m0 + 128],
                                z_bf[:, vc * BL:(vc + 1) * BL],
                                start=(vc == 2 * pair),
                                stop=(vc == 2 * pair + 1),
                            )
            for half in range(2):
                # --- LSTM cell on this hidden half; gate order [i, f, o, g]
                hsl = slice(half * HB, (half + 1) * HB)
                nc.vector.tensor_tensor(
                    gall[:, :, hsl], gh_ps[:, :, hsl],
                    gb_sb[:, :].rearrange("p (g c) -> p g c", g=4)[:, :, hsl],
                    op=OP.add,
                )
                nc.vector.tensor_tensor(
                    gall[:, :, hsl], g_ps0[:, :, hsl], gall[:, :, hsl], op=OP.add,
                )
                nc.vector.tensor_tensor(
                    gall[:, :, hsl], g_ps1[:, :, hsl], gall[:, :, hsl], op=OP.add,
                )
                # sigmoid-free cell: th = tanh(x/2), sigmoid(x) = (1+th)/2;
                # state kept as X = 2c so no extra halving op is needed.
                nc.scalar.activation(sig[:, :, hsl], gall[:, 0:3, hsl], AF.Tanh,
                                     bias=0.0, scale=0.5)
                nc.scalar.activation(tg[:, hsl], gall[:, 3, hsl], AF.Tanh)
                nc.vector.scalar_tensor_tensor(
                    t1[:, hsl], sig[:, 0, hsl], 1.0, tg[:, hsl],
                    op0=OP.add, op1=OP.mult)                 # (1+th_i)·tanh(g)
                nc.vector.scalar_tensor_tensor(
                    t2[:, hsl], sig[:, 1, hsl], 1.0, c_prev[:, hsl],
                    op0=OP.add, op1=OP.mult)                 # (1+th_f)·X_prev
                nc.vector.scalar_tensor_tensor(
                    c_f[:, hsl], t2[:, hsl], 0.5, t1[:, hsl],
                    op0=OP.mult, op1=OP.add)                 # X = 2*c_new
                nc.scalar.activation(tc_f[:, hsl], c_f[:, hsl], AF.Tanh,
                                     bias=0.0, scale=0.5)    # tanh(c_new)
                nc.vector.tensor_scalar(
                    sig[:, 0, hsl], sig[:, 2, hsl], 1.0, 0.5,
                    op0=OP.add, op1=OP.mult)                 # sigmoid(o)
                nc.vector.tensor_mul(h_bf[:, hsl], sig[:, 0, hsl], tc_f[:, hsl])

            prev_h = h_bf

        emit_head(STEPS - 1, h_bf)
        nc.sync.dma_start(out=outs_p[:, :], in_=outs_sb[:, :])

    nc.compile()
    return nc


def _prep_inputs(inputs):
    ins = {k: np.asarray(v, np.float32) for k, v in inputs.items()}
    eps = 1e-5
    scale = ins["bn_gamma"] / np.sqrt(ins["bn_var"] + eps)
    wc = ins["inconv_w"] * scale[:, None]                     # [512, 2048]
    bc = ins["inconv_b"] * scale + ins["bn_beta"] - ins["bn_mean"] * scale

    def pack_cols(v, ncol):  # [ncol*128] -> [128, ncol]
        return _f32(v.reshape(ncol, 128).T)

    def reord(w):  # [4096, ...] pytorch i,f,g,o -> i,f,o,g
        return np.concatenate([w[:H], w[H:2 * H], w[3 * H:], w[2 * H:3 * H]], axis=0)

    bb = reord(ins["b_ih"] + ins["b_hh"])                     # [4096]
    # gb[p, g*HC*BL + c*BL + b] = bb[g*1024 + c*128 + p]
    gb = np.repeat(bb.reshape(4, HC, 128).transpose(2, 0, 1).reshape(128, 4 * HC),
                   BL, axis=1)

    oneh = np.zeros((BL, BL * 128), np.float32)
    for b in range(BL):
        oneh[b, b * 128:(b + 1) * 128] = 1.0

    shared = {
        "convT": _bf(wc.T),
        "cb": pack_cols(bc, OC),
        "e1T": _bf(ins["e1_w"].T),
        "hs1T": _bf(ins["hs1_w"].T),
        "hc1T": _bf(ins["hc1_w"].T),
        "hsb": pack_cols(ins["hs1_b"], HC),
        "hcb": pack_cols(ins["hc1_b"], HC),
        "eh1T": _bf(ins["eh1_w"].T),
        "eh1b": _bf(ins["eh1_b"][None, :]),
        "eh3": _bf(ins["eh3_w"].reshape(OC, 128).T),
        "wcatT": _bf(reord(np.concatenate([ins["w_ih"], ins["w_hh"]], axis=1)).T),
        "gb": _f32(gb),
        "reg1T": _bf(ins["reg1_w"].T),
        "r1b": pack_cols(ins["reg1_b"], RG // 128),
        "reg4": _bf(ins["reg4_w"].reshape(RG // 128, 128).T),
        "oneh": _bf(oneh),
    }

    x = ins["x"].reshape(B, D, AN)
    in_maps = []
    for c in range(NC):
        xs = x[c * BL:(c + 1) * BL]                 # [8, 2048, 196]
        xs = xs.transpose(1, 0, 2).reshape(D, NF)   # [2048, 1568]
        m = dict(shared)
        m["x"] = _bf(xs)
        in_maps.append(m)
    return in_maps, float(ins["reg4_b"])


def _install_trace_hook():
    """The agent image's antenv lacks axon_hooks; synthesize it so
    run_bass_kernel_spmd(trace=True) can reach NTFF profiling."""
    import types

    try:
        from antenv.axon_hooks import get_axon_ntff_profile_hook  # noqa: F401
        return
    except ImportError:
        pass
    import antenv

    mod = types.ModuleType("antenv.axon_hooks")
    _h = [None]
    mod.set_axon_ntff_profile_hook = lambda h: _h.__setitem__(0, h)
    mod.get_axon_ntff_profile_hook = lambda: _h[0]
    sys.modules["antenv.axon_hooks"] = mod
    antenv.axon_hooks = mod
    sys.path.insert(0, "/root/.axon_site")
    from trn_agent_boot.trn_boot import _ntff_profile_via_ctypes

    hook = _ntff_profile_via_ctypes("/opt/axon/libaxon_pjrt.so")
    if hook is not None:
        mod.set_axon_ntff_profile_hook(hook)


def kernel(**inputs):
    global LAST_RESULT
    if "nc" not in _CACHE:
        _CACHE["W0"] = float(
            np.asarray(inputs["eh3_w"], np.float32)
            .astype(ml_dtypes.bfloat16).astype(np.float32).sum())
        _CACHE["nc"] = build_nc(_CACHE["W0"])
    if bool(int(os.environ.get("KERNEL_TRACE", "0"))):
        _install_trace_hook()
    nc = _CACHE["nc"]
    in_maps, reg4_b = _prep_inputs(inputs)
    res = run_bass_kernel_spmd(
        nc, in_maps, core_ids=list(range(NC)),
        trace=bool(int(os.environ.get("KERNEL_TRACE", "0"))),
    )
    LAST_RESULT = res
    out_seq = np.zeros((B, STEPS), np.float32)
    alphas = np.zeros((B, STEPS, AN), np.float32)
    for c in range(NC):
        r = res.results[c]
        out_seq[c * BL:(c + 1) * BL] = r["outs"].reshape(STEPS, BL).T + reg4_b
        alphas[c * BL:(c + 1) * BL] = np.asarray(r["alphas"], np.float32)
    return out_seq, alphas


# revision 32
# speedup vs baseline: 1.2803x; 1.0889x over previous
"""AMemNet (conv -> attention-LSTM) Trainium2 kernel, 8-core data parallel.

Shapes: B=64, D=2048, spatial 14x14=196, AV=512, H=1024, RG=512, STEPS=8.
Sharding: batch 64 -> 8 per core; all weights replicated.
"""

import os
import sys

import numpy as np

sys.path.insert(0, "/opt/trn_rl_repo")

import ml_dtypes  # noqa: E402
from contextlib import ExitStack  # noqa: E402

import concourse.bass as bass  # noqa: E402
import concourse.bacc as bacc  # noqa: E402
import concourse.tile as tile  # noqa: E402
from concourse import mybir  # noqa: E402
from concourse.bass_utils import run_bass_kernel_spmd  # noqa: E402

B, D, AV, AN, H, RG, STEPS = 64, 2048, 512, 196, 1024, 512, 8
NC = 8          # cores
BL = B // NC    # local batch = 8
NF = BL * AN    # 1568 free columns (batch-major x spatial)
KD = D // 128   # 16 k-chunks for conv
OC = AV // 128  # 4 output chunks (v-chunks)
HC = H // 128   # 8 hidden chunks
KC12 = (AV + H) // 128  # 12 k-chunks for gates
NK = 4          # free-dim split for psum: 4 x 392
NW = NF // NK   # 392

F32 = mybir.dt.float32
BF16 = mybir.dt.bfloat16
AF = mybir.ActivationFunctionType
OP = mybir.AluOpType

LAST_RESULT = None
_CACHE = {}


def _bf(x):
    return np.ascontiguousarray(x.astype(ml_dtypes.bfloat16))


def _f32(x):
    return np.ascontiguousarray(x.astype(np.float32))


def build_nc(W0):
    nc = bacc.Bacc()

    # ---------------- parameters ----------------
    x_p = nc.declare_dram_parameter("x", [D, NF], BF16, isOutput=False)
    convT_p = nc.declare_dram_parameter("convT", [D, AV], BF16, isOutput=False)
    cb_p = nc.declare_dram_parameter("cb", [128, OC], F32, isOutput=False)
    e1T_p = nc.declare_dram_parameter("e1T", [AV, AV], BF16, isOutput=False)
    hs1T_p = nc.declare_dram_parameter("hs1T", [AV, H], BF16, isOutput=False)
    hc1T_p = nc.declare_dram_parameter("hc1T", [AV, H], BF16, isOutput=False)
    hsb_p = nc.declare_dram_parameter("hsb", [128, HC], F32, isOutput=False)
    hcb_p = nc.declare_dram_parameter("hcb", [128, HC], F32, isOutput=False)
    eh1T_p = nc.declare_dram_parameter("eh1T", [H, AN], BF16, isOutput=False)
    eh1b_p = nc.declare_dram_parameter("eh1b", [1, AN], BF16, isOutput=False)
    eh3_p = nc.declare_dram_parameter("eh3", [128, OC], BF16, isOutput=False)
    wcatT_p = nc.declare_dram_parameter("wcatT", [AV + H, 4 * H], BF16, isOutput=False)
    gb_p = nc.declare_dram_parameter("gb", [128, 4 * HC * BL], F32, isOutput=False)
    reg1T_p = nc.declare_dram_parameter("reg1T", [H, RG], BF16, isOutput=False)
    r1b_p = nc.declare_dram_parameter("r1b", [128, RG // 128], F32, isOutput=False)
    reg4_p = nc.declare_dram_parameter("reg4", [128, RG // 128], BF16, isOutput=False)
    oneh_p = nc.declare_dram_parameter("oneh", [BL, BL * 128], BF16, isOutput=False)

    outs_p = nc.declare_dram_parameter("outs", [1, STEPS * BL], F32, isOutput=True)
    alphas_p = nc.declare_dram_parameter("alphas", [BL, STEPS, AN], BF16, isOutput=True)

    with ExitStack() as ctx:
        tc = ctx.enter_context(tile.TileContext(nc))

        # ------------- persistent pools -------------
        const = ctx.enter_context(tc.tile_pool(name="const", bufs=1))
        act = ctx.enter_context(tc.tile_pool(name="act", bufs=1))
        sc = ctx.enter_context(tc.tile_pool(name="scratch", bufs=1))
        sm = ctx.enter_context(tc.tile_pool(name="small", bufs=2))

        # small constants (sync queue, cheap)
        cb_sb = const.tile([128, OC], F32)
        nc.sync.dma_start(out=cb_sb[:, :], in_=cb_p[:, :])
        eh1b_sb = const.tile([1, AN], BF16)
        nc.sync.dma_start(out=eh1b_sb[:, :], in_=eh1b_p[:, :])
        eh3_sb = const.tile([128, OC], BF16)
        nc.sync.dma_start(out=eh3_sb[:, :], in_=eh3_p[:, :])
        gb_sb = const.tile([128, 4 * HC * BL], F32)
        nc.sync.dma_start(out=gb_sb[:, :], in_=gb_p[:, :])
        r1b_sb = const.tile([128, RG // 128], F32)
        nc.sync.dma_start(out=r1b_sb[:, :], in_=r1b_p[:, :])
        reg4_sb = const.tile([128, RG // 128], BF16)
        nc.sync.dma_start(out=reg4_sb[:, :], in_=reg4_p[:, :])
        oneh_sb = const.tile([BL, BL * 128], BF16)
        nc.sync.dma_start(out=oneh_sb[:, :], in_=oneh_p[:, :])
        hsb_sb = const.tile([128, HC], F32)
        nc.sync.dma_start(out=hsb_sb[:, :], in_=hsb_p[:, :])
        hcb_sb = const.tile([128, HC], F32)
        nc.sync.dma_start(out=hcb_sb[:, :], in_=hcb_p[:, :])
        ones_sb = const.tile([1, BL], BF16)
        nc.vector.memset(ones_sb[:, :], 1.0)
        warm_sb = const.tile([128, 512], BF16)
        nc.vector.memset(warm_sb[:, :], 0.5)
        ones128_sb = const.tile([1, 128], BF16)
        nc.vector.memset(ones128_sb[:, :], 1.0)
        zrhs_sb = const.tile([1, 4 * HC * BL], BF16)
        nc.vector.memset(zrhs_sb[:, :], 0.0)
        e1T_sb = const.tile([128, OC, AV], BF16)
        for k in range(OC):
            nc.sync.dma_start(out=e1T_sb[:, k, :], in_=e1T_p[k * 128:(k + 1) * 128, :])
        # loop-phase weights go on the gpsimd (SWDGE) queue so they do not
        # block the conv-phase x loads on the sync HWDGE queue
        eh1T_sb = const.tile([128, HC, AN], BF16)
        for k in range(HC):
            nc.gpsimd.dma_start(out=eh1T_sb[:, k, :], in_=eh1T_p[k * 128:(k + 1) * 128, :])
        reg1T_sb = const.tile([128, HC, RG], BF16)
        for k in range(HC):
            nc.gpsimd.dma_start(out=reg1T_sb[:, k, :], in_=reg1T_p[k * 128:(k + 1) * 128, :])

        # activations that persist across the whole kernel
        a_sb = act.tile([128, OC, NF], BF16)       # conv output (relu'd)
        e_sb = act.tile([128, OC, NF], BF16)       # e = relu(e1 @ a)
        af_bf = act.tile([128, OC * BL], BF16)     # pooled features [v, (vc,b)]
        outs_sb = act.tile([1, STEPS * BL], F32)

        # ================= stage 1+2: conv + relu + mean, e1 =================
        with ExitStack() as cctx:
            xpool = cctx.enter_context(tc.tile_pool(name="xp", bufs=1))
            cvw = cctx.enter_context(tc.tile_pool(name="cvw", bufs=1))
            cvps = cctx.enter_context(tc.tile_pool(name="cvps", bufs=2, space="PSUM"))

            xts = []
            wT_sb = cvw.tile([128, KD, AV], BF16)
            last_x_dma = None
            for k in range(KD):
                xt = xpool.tile([128, NF], BF16, tag=f"x{k}")
                last_x_dma = nc.sync.dma_start(out=xt[:, :], in_=x_p[k * 128:(k + 1) * 128, :])
                xts.append(xt)
                nc.sync.dma_start(out=wT_sb[:, k, :], in_=convT_p[k * 128:(k + 1) * 128, :])

            af_f = cvw.tile([128, OC * BL], F32)

            # warm the PE clock (HAM) while the x DMAs stream in
            warm_ps = cvps.tile([128, NK, 512], F32, tag="cvbig")
            for w in range(24):
                nc.tensor.matmul(
                    warm_ps[:, w % NK, :],
                    warm_sb[:, 0:128], warm_sb[:, :],
                    start=True, stop=True,
                )

            for oc in range(OC):
                a_ps = cvps.tile([128, NK, 512], F32, tag="cvbig")
                for k in range(KD):
                    for nw in range(NK):
                        nc.tensor.matmul(
                            a_ps[:, nw, :NW],
                            wT_sb[:, k, oc * 128:(oc + 1) * 128],
                            xts[k][:, nw * NW:(nw + 1) * NW],
                            start=(k == 0),
                            stop=(k == KD - 1),
                        )
                for nw in range(NK):
                    nc.scalar.activation(
                        a_sb[:, oc, nw * NW:(nw + 1) * NW], a_ps[:, nw, :NW],
                        AF.Relu, bias=cb_sb[:, oc:oc + 1], scale=1.0,
                    )
                for b in range(BL):
                    nc.vector.reduce_sum(
                        af_f[:, oc * BL + b:oc * BL + b + 1],
                        a_sb[:, oc, b * AN:(b + 1) * AN],
                        axis=mybir.AxisListType.X,
                    )
            nc.scalar.activation(af_bf[:, :], af_f[:, :], AF.Copy, bias=0.0, scale=1.0 / AN)

            # e = relu(e1 @ a)
            for mc in range(OC):
                e_ps = cvps.tile([128, NK, 512], F32, tag="cvbig")
                for k in range(OC):
                    for nw in range(NK):
                        nc.tensor.matmul(
                            e_ps[:, nw, :NW],
                            e1T_sb[:, k, mc * 128:(mc + 1) * 128],
                            a_sb[:, k, nw * NW:(nw + 1) * NW],
                            start=(k == 0),
                            stop=(k == OC - 1),
                        )
                for nw in range(NK):
                    nc.scalar.activation(
                        e_sb[:, mc, nw * NW:(nw + 1) * NW], e_ps[:, nw, :NW], AF.Relu,
                    )

        # ============ loop-phase pools (conv psum/sbuf freed above) ============
        ps = ctx.enter_context(tc.tile_pool(name="ps", bufs=1, space="PSUM"))
        wcp = ctx.enter_context(tc.tile_pool(name="wcp", bufs=1))
        wcat_sb = wcp.tile([128, KC12, 4 * H], BF16)
        from bass_rust import add_dep_helper
        for k in range(KC12):
            wdma = nc.gpsimd.dma_start(out=wcat_sb[:, k, :], in_=wcatT_p[k * 128:(k + 1) * 128, :])
            if k == 0 and last_x_dma is not None:
                add_dep_helper(wdma.ins, last_x_dma.ins,
                               reason="hold wcat until x loads finish")

        # ============ stage 3: h0 = tanh(hs1@af), c0 = tanh(hc1@af) ========
        h_bf = sm.tile([128, HC * BL], BF16, tag="h")
        c_f = sm.tile([128, HC * BL], F32, tag="c")
        for src_p, bias_sb, dst, dt in (
            (hs1T_p, hsb_sb, h_bf, AF.Tanh),
            (hc1T_p, hcb_sb, c_f, AF.Tanh),
        ):
            winit = sc.tile([128, OC, H], BF16, tag="winit")
            for k in range(OC):
                nc.sync.dma_start(out=winit[:, k, :], in_=src_p[k * 128:(k + 1) * 128, :])
            for mc in range(HC):
                h_ps = ps.tile([128, BL], F32, tag="sm8", bufs=3)
                for k in range(OC):
                    nc.tensor.matmul(
                        h_ps[:, :],
                        winit[:, k, mc * 128:(mc + 1) * 128],
                        af_bf[:, k * BL:(k + 1) * BL],
                        start=(k == 0),
                        stop=(k == OC - 1),
                    )
                nc.scalar.activation(
                    dst[:, mc * BL:(mc + 1) * BL], h_ps[:, :],
                    dt, bias=bias_sb[:, mc:mc + 1], scale=1.0,
                )
            if dst is c_f:
                nc.vector.tensor_scalar_mul(c_f[:, :], c_f[:, :], 2.0)

        # ================= stage 4: recurrent attention loop =================
        prev_h = None

        def emit_head(tt, h_src):
            # out_tt = relu(reg1 @ h + b) . reg4
            r_bf = sm.tile([128, (RG // 128) * BL], BF16, tag="rbf", bufs=1)
            for mc in range(RG // 128):
                r_ps = ps.tile([128, BL], F32, tag="sm8", bufs=3)
                for k in range(HC):
                    nc.tensor.matmul(
                        r_ps[:, :],
                        reg1T_sb[:, k, mc * 128:(mc + 1) * 128],
                        h_src[:, k * BL:(k + 1) * BL],
                        start=(k == 0),
                        stop=(k == HC - 1),
                    )
                nc.scalar.activation(
                    r_bf[:, mc * BL:(mc + 1) * BL], r_ps[:, :],
                    AF.Relu, bias=r1b_sb[:, mc:mc + 1], scale=1.0,
                )
            o_ps = ps.tile([1, BL], F32, tag="sm8", bufs=3)
            for mc in range(RG // 128):
                nc.tensor.matmul(
                    o_ps[:, :],
                    reg4_sb[:, mc:mc + 1],
                    r_bf[:, mc * BL:(mc + 1) * BL],
                    start=(mc == 0),
                    stop=(mc == RG // 128 - 1),
                )
            nc.scalar.activation(outs_sb[:, tt * BL:(tt + 1) * BL], o_ps[:, :], AF.Copy)

        # Gate order in wcat/gb is host-reordered to [i, f, o, g] so one
        # fused Sigmoid covers gall[:, 0:3, :].
        # relu(e+eh) = max(e, -eh) + eh: the +eh term is constant across v,
        # so logits = sum_v w_v * max(e, -eh)  +  W0 * eh  (W0 = sum_v w_v).
        def gh_slice(g_lo, g_hi, gh_ps, h_src):
            for g in range(g_lo, g_hi):
                for hcj in range(HC):
                    m0 = g * H + hcj * 128
                    for k in range(OC, KC12):
                        nc.tensor.matmul(
                            gh_ps[:, g, hcj * BL:(hcj + 1) * BL],
                            wcat_sb[:, k, m0:m0 + 128],
                            h_src[:, (k - OC) * BL:(k - OC + 1) * BL],
                            start=False,
                            stop=False,
                            skip_group_check=True,
                        )

        for t in range(STEPS):
            # --- eh[b, n] = (h @ eh1_w.T + eh1_b), directly as [BL, AN]
            eh_ps = ps.tile([BL, 256], F32, tag="sm8", bufs=3)
            for k in range(HC):
                nc.tensor.matmul(
                    eh_ps[:, :AN],
                    h_bf[:, k * BL:(k + 1) * BL],
                    eh1T_sb[:, k, :],
                    start=(k == 0),
                    stop=False,
                )
            nc.tensor.matmul(
                eh_ps[:, :AN], ones_sb[:, :], eh1b_sb[:, :], start=False, stop=True,
            )
            nehT_sb = sm.tile([BL, AN], BF16, tag="ehT", bufs=1)   # -eh
            nc.scalar.activation(nehT_sb[:, :], eh_ps[:, :AN], AF.Copy, bias=0.0, scale=-1.0)

            if prev_h is not None:
                emit_head(t - 1, prev_h)

            # --- broadcast -eh across partitions -> SBUF bf16
            nehb_sb = sc.tile([128, BL, AN], BF16, tag="bcast", bufs=1)
            for j in range(4):
                bc_ps = ps.tile([128, 2, AN], F32, tag=f"bc{j}", bufs=1)
                for i in range(2):
                    b = 2 * j + i
                    nc.tensor.matmul(
                        bc_ps[:, i, :],
                        oneh_sb[:, b * 128:(b + 1) * 128],
                        nehT_sb[:, :],
                        start=True,
                        stop=True,
                    )
                nc.scalar.activation(nehb_sb[:, 2 * j:2 * j + 2, :], bc_ps[:, :, :], AF.Copy)

            gh_ps = ps.tile([128, 4, HC * BL], F32, tag="gatesh", bufs=1)
            nc.tensor.matmul(
                gh_ps[:, :, :].rearrange("p g c -> p (g c)"),
                ones128_sb[:, :], zrhs_sb[:, :],
                start=True, stop=False, skip_group_check=True,
            )

            # --- m = max(e, -ehb)
            s_sb = sc.tile([128, OC, NF], BF16, tag="s", bufs=1)
            for vc in range(OC):
                nc.vector.tensor_tensor(
                    s_sb[:, vc, :], e_sb[:, vc, :],
                    nehb_sb[:, :, :].rearrange("p b n -> p (b n)"),
                    op=OP.max,
                )

            # --- logits_m = sum_v eh3_w[v] * m[v, :]
            lg_sb = sm.tile([1, NF], F32, tag="lg", bufs=1)
            for nk in range(NK):
                lg_ps = ps.tile([1, NW], F32, tag="sm8", bufs=3)
                for vc in range(OC):
                    nc.tensor.matmul(
                        lg_ps[:, :],
                        eh3_sb[:, vc:vc + 1],
                        s_sb[:, vc, nk * NW:(nk + 1) * NW],
                        start=(vc == 0),
                        stop=(vc == OC - 1),
                    )
                nc.scalar.activation(lg_sb[:, nk * NW:(nk + 1) * NW], lg_ps[:, :], AF.Copy)

            # --- gates h-part slice A (dense PE block; hides softmax chain)
            gh_slice(0, 2, gh_ps, h_bf)

            # --- reshape logits, add back W0*eh, softmax over n
            lgT = sm.tile([BL, AN], F32, tag="lgT", bufs=1)
            nc.sync.dma_start(out=lgT[:, :], in_=lg_sb[:, :])
            nc.vector.scalar_tensor_tensor(
                lgT[:, :], nehT_sb[:, :], -W0, lgT[:, :],
                op0=OP.mult, op1=OP.add,
            )
            exp_sb = sm.tile([BL, AN], F32, tag="exp", bufs=1)
            sumx = sm.tile([BL, 1], F32, tag="sumx", bufs=1)
            nc.scalar.activation(exp_sb[:, :], lgT[:, :], AF.Exp, accum_out=sumx[:, :])
            rcp = sm.tile([BL, 1], F32, tag="rcp", bufs=1)
            nc.vector.reciprocal(rcp[:, :], sumx[:, :])
            alpha_bf = sm.tile([BL, AN], BF16, tag="alphab", bufs=1)
            nc.vector.tensor_scalar_mul(alpha_bf[:, :], exp_sb[:, :], rcp[:, :])
            nc.sync.dma_start(out=alphas_p[:, t, :], in_=alpha_bf[:, :])

            # --- broadcast alpha -> SBUF bf16
            alb_sb = sc.tile([128, BL, AN], BF16, tag="bcast", bufs=1)
            for j in range(4):
                bc_ps = ps.tile([128, 2, AN], F32, tag=f"bc{j}", bufs=1)
                for i in range(2):
                    b = 2 * j + i
                    nc.tensor.matmul(
                        bc_ps[:, i, :],
                        oneh_sb[:, b * 128:(b + 1) * 128],
                        alpha_bf[:, :],
                        start=True,
                        stop=True,
                    )
                nc.scalar.activation(alb_sb[:, 2 * j:2 * j + 2, :], bc_ps[:, :, :], AF.Copy)

            # --- gates h-part slice B (hides z mul/reduce)
            gh_slice(2, 4, gh_ps, h_bf)

            # --- z = sum_n a*alb, then gates z-part (window-contiguous groups:
            #     start=True clears has_written for the whole bank, so a
            #     window's accumulation may not interleave with another's)

            z_f = sm.tile([128, OC * BL], F32, tag="zf", bufs=1)
            z_bf = sm.tile([128, OC * BL], BF16, tag="zbf", bufs=1)
            hs_sb = sm.tile([128, BL * (AN // 2)], BF16, tag="hsum", bufs=1)
            for vc in range(OC):
                nc.vector.tensor_tensor(
                    s_sb[:, vc, :], a_sb[:, vc, :],
                    alb_sb[:, :, :].rearrange("p b n -> p (b n)"),
                    op=OP.mult,
                )
                pv = s_sb[:, vc, :].rearrange("p (b h n) -> p b h n", b=BL, h=2)
                nc.vector.tensor_tensor(
                    hs_sb[:, :].rearrange("p (b n) -> p b n", b=BL),
                    pv[:, :, 0, :], pv[:, :, 1, :], op=OP.add,
                )
                nc.vector.reduce_sum(
                    z_f[:, vc * BL:(vc + 1) * BL],
                    hs_sb[:, :].rearrange("p (b n) -> p b n", b=BL),
                    axis=mybir.AxisListType.X,
                )
                nc.vector.tensor_copy(
                    z_bf[:, vc * BL:(vc + 1) * BL], z_f[:, vc * BL:(vc + 1) * BL])
            c_prev = c_f
            gall = sm.tile([128, 4, HC * BL], F32, tag="gall", bufs=1)
            sig = sm.tile([128, 3, HC * BL], F32, tag="sig", bufs=1)
            tg = sm.tile([128, HC * BL], F32, tag="tg", bufs=1)
            t1 = sm.tile([128, HC * BL], F32, tag="t1", bufs=1)
            t2 = sm.tile([128, HC * BL], F32, tag="t2", bufs=1)
            c_f = sm.tile([128, HC * BL], F32, tag="c")
            tc_f = sm.tile([128, HC * BL], F32, tag="tc", bufs=1)
            h_bf = sm.tile([128, HC * BL], BF16, tag="h")
            HB = HC * BL // 2
            # z-part accumulates straight onto gh_ps (bank pre-zeroed, so no
            # start=True anywhere -> groups need not be contiguous) k-OUTER:
            # each z chunk is consumed by 32 matmuls as soon as it reduces.
            for vc in range(OC):
                for g in range(4):
                    for hcj in range(HC):
                        m0 = g * H + hcj * 128
                        nc.tensor.matmul(
                            gh_ps[:, g, hcj * BL:(hcj + 1) * BL],
                            wcat_sb[:, vc, m0:m0 + 128],
                            z_bf[:, vc * BL:(vc + 1) * BL],
                            start=False,
                            stop=False,
                            skip_group_check=True,
                        )
            # --- LSTM cell (full width); gate order [i, f, o, g];
            #     sigmoid-free: th = tanh(x/2), sigmoid(x) = (1+th)/2,
            #     state kept as X = 2c.
            nc.vector.tensor_tensor(
                gall[:, :, :], gh_ps[:, :, :],
                gb_sb[:, :].rearrange("p (g c) -> p g c", g=4),
                op=OP.add,
            )
            nc.scalar.activation(sig[:, :, :], gall[:, 0:3, :], AF.Tanh,
                                 bias=0.0, scale=0.5)
            nc.scalar.activation(tg[:, :], gall[:, 3, :], AF.Tanh)
            nc.vector.scalar_tensor_tensor(
                t1[:, :], sig[:, 0, :], 1.0, tg[:, :],
                op0=OP.add, op1=OP.mult)                 # (1+th_i)·tanh(g)
            nc.vector.scalar_tensor_tensor(
                t2[:, :], sig[:, 1, :], 1.0, c_prev[:, :],
                op0=OP.add, op1=OP.mult)                 # (1+th_f)·X_prev
            nc.vector.scalar_tensor_tensor(
                c_f[:, :], t2[:, :], 0.5, t1[:, :],
                op0=OP.mult, op1=OP.add)                 # X = 2*c_new
            nc.scalar.activation(tc_f[:, :], c_f[:, :], AF.Tanh,
                                 bias=0.0, scale=0.5)    # tanh(c_new)
            nc.vector.tensor_scalar(
                sig[:, 0, :], sig[:, 2, :], 1.0, 0.5,
                op0=OP.add, op1=OP.mult)                 # sigmoid(o)
            nc.vector.tensor_mul(h_bf[:, :], sig[:, 0, :], tc_f[:, :])

            prev_h = h_bf

        emit_head(STEPS - 1, h_bf)
        nc.sync.dma_start(out=outs_p[:, :], in_=outs_sb[:, :])

    nc.compile()
    return nc


def _prep_inputs(inputs):
    ins = {k: np.asarray(v, np.float32) for k, v in inputs.items()}
    eps = 1e-5
    scale = ins["bn_gamma"] / np.sqrt(ins["bn_var"] + eps)
    wc = ins["inconv_w"] * scale[:, None]                     # [512, 2048]
    bc = ins["inconv_b"] * scale + ins["bn_beta"] - ins["bn_mean"] * scale

    def pack_cols(v, ncol):  # [ncol*128] -> [128, ncol]
        return _f32(v.reshape(ncol, 128).T)

    def reord(w):  # [4096, ...] pytorch i,f,g,o -> i,f,o,g
        return np.concatenate([w[:H], w[H:2 * H], w[3 * H:], w[2 * H:3 * H]], axis=0)

    bb = reord(ins["b_ih"] + ins["b_hh"])                     # [4096]
    # gb[p, g*HC*BL + c*BL + b] = bb[g*1024 + c*128 + p]
    gb = np.repeat(bb.reshape(4, HC, 128).transpose(2, 0, 1).reshape(128, 4 * HC),
                   BL, axis=1)

    oneh = np.zeros((BL, BL * 128), np.float32)
    for b in range(BL):
        oneh[b, b * 128:(b + 1) * 128] = 1.0

    shared = {
        "convT": _bf(wc.T),
        "cb": pack_cols(bc, OC),
        "e1T": _bf(ins["e1_w"].T),
        "hs1T": _bf(ins["hs1_w"].T),
        "hc1T": _bf(ins["hc1_w"].T),
        "hsb": pack_cols(ins["hs1_b"], HC),
        "hcb": pack_cols(ins["hc1_b"], HC),
        "eh1T": _bf(ins["eh1_w"].T),
        "eh1b": _bf(ins["eh1_b"][None, :]),
        "eh3": _bf(ins["eh3_w"].reshape(OC, 128).T),
        "wcatT": _bf(reord(np.concatenate([ins["w_ih"], ins["w_hh"]], axis=1)).T),
        "gb": _f32(gb),
        "reg1T": _bf(ins["reg1_w"].T),
        "r1b": pack_cols(ins["reg1_b"], RG // 128),
        "reg4": _bf(ins["reg4_w"].reshape(RG // 128, 128).T),
        "oneh": _bf(oneh),
    }

    x = ins["x"].reshape(B, D, AN)
    in_maps = []
    for c in range(NC):
        xs = x[c * BL:(c + 1) * BL]                 # [8, 2048, 196]
        xs = xs.transpose(1, 0, 2).reshape(D, NF)   # [2048, 1568]
        m = dict(shared)
        m["x"] = _bf(xs)
        in_maps.append(m)
    return in_maps, float(ins["reg4_b"])


def _install_trace_hook():
    """The agent image's antenv lacks axon_hooks; synthesize it so
    run_bass_kernel_spmd(trace=True) can reach NTFF profiling."""
    import types

    try:
        from antenv.axon_hooks import get_axon_ntff_profile_hook  # noqa: F401
        return
    except ImportError:
        pass
    import antenv

    mod = types.ModuleType("antenv.axon_hooks")
    _h = [None]
    mod.set_axon_ntff_profile_hook = lambda h: _h.__setitem__(0, h)
    mod.get_axon_ntff_profile_hook = lambda: _h[0]
    sys.modules["antenv.axon_hooks"] = mod
    antenv.axon_hooks = mod
    sys.path.insert(0, "/root/.axon_site")
    from trn_agent_boot.trn_boot import _ntff_profile_via_ctypes

    hook = _ntff_profile_via_ctypes("/opt/axon/libaxon_pjrt.so")
    if hook is not None:
        mod.set_axon_ntff_profile_hook(hook)


def kernel(**inputs):
    global LAST_RESULT
    if "nc" not in _CACHE:
        _CACHE["W0"] = float(
            np.asarray(inputs["eh3_w"], np.float32)
            .astype(ml_dtypes.bfloat16).astype(np.float32).sum())
        _CACHE["nc"] = build_nc(_CACHE["W0"])
    if bool(int(os.environ.get("KERNEL_TRACE", "0"))):
        _install_trace_hook()
    nc = _CACHE["nc"]
    in_maps, reg4_b = _prep_inputs(inputs)
    res = run_bass_kernel_spmd(
        nc, in_maps, core_ids=list(range(NC)),
        trace=bool(int(os.environ.get("KERNEL_TRACE", "0"))),
    )
    LAST_RESULT = res
    out_seq = np.zeros((B, STEPS), np.float32)
    alphas = np.zeros((B, STEPS, AN), np.float32)
    for c in range(NC):
        r = res.results[c]
        out_seq[c * BL:(c + 1) * BL] = r["outs"].reshape(STEPS, BL).T + reg4_b
        alphas[c * BL:(c + 1) * BL] = np.asarray(r["alphas"], np.float32)
    return out_seq, alphas


# revision 33
# speedup vs baseline: 1.3269x; 1.0364x over previous
"""AMemNet (conv -> attention-LSTM) Trainium2 kernel, 8-core data parallel.

Shapes: B=64, D=2048, spatial 14x14=196, AV=512, H=1024, RG=512, STEPS=8.
Sharding: batch 64 -> 8 per core; all weights replicated.
"""

import os
import sys

import numpy as np

sys.path.insert(0, "/opt/trn_rl_repo")

import ml_dtypes  # noqa: E402
from contextlib import ExitStack  # noqa: E402

import concourse.bass as bass  # noqa: E402
import concourse.bacc as bacc  # noqa: E402
import concourse.tile as tile  # noqa: E402
from concourse import mybir  # noqa: E402
from concourse.bass_utils import run_bass_kernel_spmd  # noqa: E402

B, D, AV, AN, H, RG, STEPS = 64, 2048, 512, 196, 1024, 512, 8
NC = 8          # cores
BL = B // NC    # local batch = 8
NF = BL * AN    # 1568 free columns (batch-major x spatial)
KD = D // 128   # 16 k-chunks for conv
OC = AV // 128  # 4 output chunks (v-chunks)
HC = H // 128   # 8 hidden chunks
KC12 = (AV + H) // 128  # 12 k-chunks for gates
NK = 4          # free-dim split for psum: 4 x 392
NW = NF // NK   # 392

F32 = mybir.dt.float32
BF16 = mybir.dt.bfloat16
AF = mybir.ActivationFunctionType
OP = mybir.AluOpType

LAST_RESULT = None
_CACHE = {}


def _bf(x):
    return np.ascontiguousarray(x.astype(ml_dtypes.bfloat16))


def _f32(x):
    return np.ascontiguousarray(x.astype(np.float32))


def build_nc(W0):
    nc = bacc.Bacc()

    # ---------------- parameters ----------------
    x_p = nc.declare_dram_parameter("x", [D, NF], BF16, isOutput=False)
    convT_p = nc.declare_dram_parameter("convT", [D, AV], BF16, isOutput=False)
    cb_p = nc.declare_dram_parameter("cb", [128, OC], F32, isOutput=False)
    e1T_p = nc.declare_dram_parameter("e1T", [AV, AV], BF16, isOutput=False)
    hs1T_p = nc.declare_dram_parameter("hs1T", [AV, H], BF16, isOutput=False)
    hc1T_p = nc.declare_dram_parameter("hc1T", [AV, H], BF16, isOutput=False)
    hsb_p = nc.declare_dram_parameter("hsb", [128, HC], F32, isOutput=False)
    hcb_p = nc.declare_dram_parameter("hcb", [128, HC], F32, isOutput=False)
    eh1T_p = nc.declare_dram_parameter("eh1T", [H, AN], BF16, isOutput=False)
    eh1b_p = nc.declare_dram_parameter("eh1b", [1, AN], BF16, isOutput=False)
    eh3_p = nc.declare_dram_parameter("eh3", [128, OC], BF16, isOutput=False)
    wcatT_p = nc.declare_dram_parameter("wcatT", [AV + H, 4 * H], BF16, isOutput=False)
    gb_p = nc.declare_dram_parameter("gb", [128, 4 * HC * BL], F32, isOutput=False)
    reg1T_p = nc.declare_dram_parameter("reg1T", [H, RG], BF16, isOutput=False)
    r1b_p = nc.declare_dram_parameter("r1b", [128, RG // 128], F32, isOutput=False)
    reg4_p = nc.declare_dram_parameter("reg4", [128, RG // 128], BF16, isOutput=False)
    oneh_p = nc.declare_dram_parameter("oneh", [BL, BL * 128], BF16, isOutput=False)

    outs_p = nc.declare_dram_parameter("outs", [1, STEPS * BL], F32, isOutput=True)
    alphas_p = nc.declare_dram_parameter("alphas", [BL, STEPS, AN], BF16, isOutput=True)

    with ExitStack() as ctx:
        tc = ctx.enter_context(tile.TileContext(nc))

        # ------------- persistent pools -------------
        const = ctx.enter_context(tc.tile_pool(name="const", bufs=1))
        act = ctx.enter_context(tc.tile_pool(name="act", bufs=1))
        sc = ctx.enter_context(tc.tile_pool(name="scratch", bufs=1))
        sm = ctx.enter_context(tc.tile_pool(name="small", bufs=2))

        # small constants (sync queue, cheap)
        cb_sb = const.tile([128, OC], F32)
        nc.sync.dma_start(out=cb_sb[:, :], in_=cb_p[:, :])
        eh1b_sb = const.tile([1, AN], BF16)
        nc.sync.dma_start(out=eh1b_sb[:, :], in_=eh1b_p[:, :])
        eh3_sb = const.tile([128, OC], BF16)
        nc.sync.dma_start(out=eh3_sb[:, :], in_=eh3_p[:, :])
        gb_sb = const.tile([128, 4 * HC * BL], F32)
        nc.sync.dma_start(out=gb_sb[:, :], in_=gb_p[:, :])
        r1b_sb = const.tile([128, RG // 128], F32)
        nc.sync.dma_start(out=r1b_sb[:, :], in_=r1b_p[:, :])
        reg4_sb = const.tile([128, RG // 128], BF16)
        nc.sync.dma_start(out=reg4_sb[:, :], in_=reg4_p[:, :])
        oneh_sb = const.tile([BL, BL * 128], BF16)
        nc.sync.dma_start(out=oneh_sb[:, :], in_=oneh_p[:, :])
        hsb_sb = const.tile([128, HC], F32)
        nc.sync.dma_start(out=hsb_sb[:, :], in_=hsb_p[:, :])
        hcb_sb = const.tile([128, HC], F32)
        nc.sync.dma_start(out=hcb_sb[:, :], in_=hcb_p[:, :])
        ones_sb = const.tile([1, BL], BF16)
        nc.vector.memset(ones_sb[:, :], 1.0)
        warm_sb = const.tile([128, 512], BF16)
        nc.vector.memset(warm_sb[:, :], 0.5)
        ones128_sb = const.tile([1, 128], BF16)
        nc.vector.memset(ones128_sb[:, :], 1.0)
        zrhs_sb = const.tile([1, 4 * HC * BL], BF16)
        nc.vector.memset(zrhs_sb[:, :], 0.0)
        e1T_sb = const.tile([128, OC, AV], BF16)
        for k in range(OC):
            nc.sync.dma_start(out=e1T_sb[:, k, :], in_=e1T_p[k * 128:(k + 1) * 128, :])
        # loop-phase weights go on the gpsimd (SWDGE) queue so they do not
        # block the conv-phase x loads on the sync HWDGE queue
        eh1T_sb = const.tile([128, HC, AN], BF16)
        for k in range(HC):
            nc.gpsimd.dma_start(out=eh1T_sb[:, k, :], in_=eh1T_p[k * 128:(k + 1) * 128, :])
        reg1T_sb = const.tile([128, HC, RG], BF16)
        for k in range(HC):
            nc.gpsimd.dma_start(out=reg1T_sb[:, k, :], in_=reg1T_p[k * 128:(k + 1) * 128, :])

        # activations that persist across the whole kernel
        a_sb = act.tile([128, OC, NF], BF16)       # conv output (relu'd)
        e_sb = act.tile([128, OC, NF], BF16)       # e = relu(e1 @ a)
        af_bf = act.tile([128, OC * BL], BF16)     # pooled features [v, (vc,b)]
        outs_sb = act.tile([1, STEPS * BL], F32)

        # ================= stage 1+2: conv + relu + mean, e1 =================
        with ExitStack() as cctx:
            xpool = cctx.enter_context(tc.tile_pool(name="xp", bufs=1))
            cvw = cctx.enter_context(tc.tile_pool(name="cvw", bufs=1))
            cvps = cctx.enter_context(tc.tile_pool(name="cvps", bufs=2, space="PSUM"))

            xts = []
            wT_sb = cvw.tile([128, KD, AV], BF16)
            last_x_dma = None
            for k in range(KD):
                xt = xpool.tile([128, NF], BF16, tag=f"x{k}")
                last_x_dma = nc.sync.dma_start(out=xt[:, :], in_=x_p[k * 128:(k + 1) * 128, :])
                xts.append(xt)
                nc.sync.dma_start(out=wT_sb[:, k, :], in_=convT_p[k * 128:(k + 1) * 128, :])

            af_f = cvw.tile([128, OC * BL], F32)

            # warm the PE clock (HAM) while the x DMAs stream in
            warm_ps = cvps.tile([128, NK, 512], F32, tag="cvbig")
            for w in range(24):
                nc.tensor.matmul(
                    warm_ps[:, w % NK, :],
                    warm_sb[:, 0:128], warm_sb[:, :],
                    start=True, stop=True,
                )

            for oc in range(OC):
                a_ps = cvps.tile([128, NK, 512], F32, tag="cvbig")
                for k in range(KD):
                    for nw in range(NK):
                        nc.tensor.matmul(
                            a_ps[:, nw, :NW],
                            wT_sb[:, k, oc * 128:(oc + 1) * 128],
                            xts[k][:, nw * NW:(nw + 1) * NW],
                            start=(k == 0),
                            stop=(k == KD - 1),
                        )
                for nw in range(NK):
                    nc.scalar.activation(
                        a_sb[:, oc, nw * NW:(nw + 1) * NW], a_ps[:, nw, :NW],
                        AF.Relu, bias=cb_sb[:, oc:oc + 1], scale=1.0,
                    )
                for b in range(BL):
                    nc.vector.reduce_sum(
                        af_f[:, oc * BL + b:oc * BL + b + 1],
                        a_sb[:, oc, b * AN:(b + 1) * AN],
                        axis=mybir.AxisListType.X,
                    )
            nc.scalar.activation(af_bf[:, :], af_f[:, :], AF.Copy, bias=0.0, scale=1.0 / AN)

            # e = relu(e1 @ a)
            for mc in range(OC):
                e_ps = cvps.tile([128, NK, 512], F32, tag="cvbig")
                for k in range(OC):
                    for nw in range(NK):
                        nc.tensor.matmul(
                            e_ps[:, nw, :NW],
                            e1T_sb[:, k, mc * 128:(mc + 1) * 128],
                            a_sb[:, k, nw * NW:(nw + 1) * NW],
                            start=(k == 0),
                            stop=(k == OC - 1),
                        )
                for nw in range(NK):
                    nc.scalar.activation(
                        e_sb[:, mc, nw * NW:(nw + 1) * NW], e_ps[:, nw, :NW], AF.Relu,
                    )

        # ============ loop-phase pools (conv psum/sbuf freed above) ============
        ps = ctx.enter_context(tc.tile_pool(name="ps", bufs=1, space="PSUM"))
        wcp = ctx.enter_context(tc.tile_pool(name="wcp", bufs=1))
        wcat_sb = wcp.tile([128, KC12, 4 * H], BF16)
        from bass_rust import add_dep_helper
        for k in range(KC12):
            wdma = nc.gpsimd.dma_start(out=wcat_sb[:, k, :], in_=wcatT_p[k * 128:(k + 1) * 128, :])
            if k == 0 and last_x_dma is not None:
                add_dep_helper(wdma.ins, last_x_dma.ins,
                               reason="hold wcat until x loads finish")

        # ============ stage 3: h0 = tanh(hs1@af), c0 = tanh(hc1@af) ========
        h_bf = sm.tile([128, HC * BL], BF16, tag="h")
        c_f = sm.tile([128, HC * BL], F32, tag="c")
        for src_p, bias_sb, dst, dt in (
            (hs1T_p, hsb_sb, h_bf, AF.Tanh),
            (hc1T_p, hcb_sb, c_f, AF.Tanh),
        ):
            winit = sc.tile([128, OC, H], BF16, tag="winit")
            for k in range(OC):
                nc.sync.dma_start(out=winit[:, k, :], in_=src_p[k * 128:(k + 1) * 128, :])
            for mc in range(HC):
                h_ps = ps.tile([128, BL], F32, tag="sm8", bufs=3)
                for k in range(OC):
                    nc.tensor.matmul(
                        h_ps[:, :],
                        winit[:, k, mc * 128:(mc + 1) * 128],
                        af_bf[:, k * BL:(k + 1) * BL],
                        start=(k == 0),
                        stop=(k == OC - 1),
                    )
                nc.scalar.activation(
                    dst[:, mc * BL:(mc + 1) * BL], h_ps[:, :],
                    dt, bias=bias_sb[:, mc:mc + 1], scale=1.0,
                )
            if dst is c_f:
                nc.vector.tensor_scalar_mul(c_f[:, :], c_f[:, :], 2.0)

        # ================= stage 4: recurrent attention loop =================
        prev_h = None

        def emit_head(tt, h_src):
            # out_tt = relu(reg1 @ h + b) . reg4
            r_bf = sm.tile([128, (RG // 128) * BL], BF16, tag="rbf", bufs=1)
            for mc in range(RG // 128):
                r_ps = ps.tile([128, BL], F32, tag="sm8", bufs=3)
                for k in range(HC):
                    nc.tensor.matmul(
                        r_ps[:, :],
                        reg1T_sb[:, k, mc * 128:(mc + 1) * 128],
                        h_src[:, k * BL:(k + 1) * BL],
                        start=(k == 0),
                        stop=(k == HC - 1),
                    )
                nc.scalar.activation(
                    r_bf[:, mc * BL:(mc + 1) * BL], r_ps[:, :],
                    AF.Relu, bias=r1b_sb[:, mc:mc + 1], scale=1.0,
                )
            o_ps = ps.tile([1, BL], F32, tag="sm8", bufs=3)
            for mc in range(RG // 128):
                nc.tensor.matmul(
                    o_ps[:, :],
                    reg4_sb[:, mc:mc + 1],
                    r_bf[:, mc * BL:(mc + 1) * BL],
                    start=(mc == 0),
                    stop=(mc == RG // 128 - 1),
                )
            nc.scalar.activation(outs_sb[:, tt * BL:(tt + 1) * BL], o_ps[:, :], AF.Copy)

        # Gate order in wcat/gb is host-reordered to [i, f, o, g] so one
        # fused Sigmoid covers gall[:, 0:3, :].
        # relu(e+eh) = max(e, -eh) + eh: the +eh term is constant across v,
        # so logits = sum_v w_v * max(e, -eh)  +  W0 * eh  (W0 = sum_v w_v).
        def gh_slice(g_lo, g_hi, gh_ps, h_src):
            for g in range(g_lo, g_hi):
                for hcj in range(HC):
                    m0 = g * H + hcj * 128
                    for k in range(OC, KC12):
                        nc.tensor.matmul(
                            gh_ps[:, g, hcj * BL:(hcj + 1) * BL],
                            wcat_sb[:, k, m0:m0 + 128],
                            h_src[:, (k - OC) * BL:(k - OC + 1) * BL],
                            start=False,
                            stop=False,
                            skip_group_check=True,
                        )

        for t in range(STEPS):
            # --- eh[b, n] = (h @ eh1_w.T + eh1_b), directly as [BL, AN]
            eh_ps = ps.tile([BL, 256], F32, tag="sm8", bufs=3)
            for k in range(HC):
                nc.tensor.matmul(
                    eh_ps[:, :AN],
                    h_bf[:, k * BL:(k + 1) * BL],
                    eh1T_sb[:, k, :],
                    start=(k == 0),
                    stop=False,
                )
            nc.tensor.matmul(
                eh_ps[:, :AN], ones_sb[:, :], eh1b_sb[:, :], start=False, stop=True,
            )
            nehT_sb = sm.tile([BL, AN], BF16, tag="ehT", bufs=1)   # -eh
            nc.scalar.activation(nehT_sb[:, :], eh_ps[:, :AN], AF.Copy, bias=0.0, scale=-1.0)

            # --- broadcast -eh across partitions -> SBUF bf16
            nehb_sb = sc.tile([128, BL, AN], BF16, tag="bcast", bufs=1)
            for j in range(4):
                bc_ps = ps.tile([128, 2, AN], F32, tag=f"bc{j}", bufs=1)
                for i in range(2):
                    b = 2 * j + i
                    nc.tensor.matmul(
                        bc_ps[:, i, :],
                        oneh_sb[:, b * 128:(b + 1) * 128],
                        nehT_sb[:, :],
                        start=True,
                        stop=True,
                    )
                nc.scalar.activation(nehb_sb[:, 2 * j:2 * j + 2, :], bc_ps[:, :, :], AF.Copy)

            gh_ps = ps.tile([128, 4, HC * BL], F32, tag="gatesh", bufs=1)
            nc.tensor.matmul(
                gh_ps[:, :, :].rearrange("p g c -> p (g c)"),
                ones128_sb[:, :], zrhs_sb[:, :],
                start=True, stop=False, skip_group_check=True,
            )

            # head of the previous step: fills the PE while ACT copies the
            # broadcast and DVE computes the max chain
            if prev_h is not None:
                emit_head(t - 1, prev_h)

            # --- m = max(e, -ehb)
            s_sb = sc.tile([128, OC, NF], BF16, tag="s", bufs=1)
            for vc in range(OC):
                nc.vector.tensor_tensor(
                    s_sb[:, vc, :], e_sb[:, vc, :],
                    nehb_sb[:, :, :].rearrange("p b n -> p (b n)"),
                    op=OP.max,
                )

            # --- logits_m = sum_v eh3_w[v] * m[v, :]
            lg_sb = sm.tile([1, NF], F32, tag="lg", bufs=1)
            for nk in range(NK):
                lg_ps = ps.tile([1, NW], F32, tag="sm8", bufs=3)
                for vc in range(OC):
                    nc.tensor.matmul(
                        lg_ps[:, :],
                        eh3_sb[:, vc:vc + 1],
                        s_sb[:, vc, nk * NW:(nk + 1) * NW],
                        start=(vc == 0),
                        stop=(vc == OC - 1),
                    )
                nc.scalar.activation(lg_sb[:, nk * NW:(nk + 1) * NW], lg_ps[:, :], AF.Copy)

            # --- gates h-part slice A (dense PE block; hides softmax chain)
            gh_slice(0, 2, gh_ps, h_bf)

            # --- reshape logits, add back W0*eh, softmax over n
            lgT = sm.tile([BL, AN], F32, tag="lgT", bufs=1)
            nc.sync.dma_start(out=lgT[:, :], in_=lg_sb[:, :])
            nc.vector.scalar_tensor_tensor(
                lgT[:, :], nehT_sb[:, :], -W0, lgT[:, :],
                op0=OP.mult, op1=OP.add,
            )
            exp_sb = sm.tile([BL, AN], F32, tag="exp", bufs=1)
            sumx = sm.tile([BL, 1], F32, tag="sumx", bufs=1)
            nc.scalar.activation(exp_sb[:, :], lgT[:, :], AF.Exp, accum_out=sumx[:, :])
            rcp = sm.tile([BL, 1], F32, tag="rcp", bufs=1)
            nc.vector.reciprocal(rcp[:, :], sumx[:, :])
            alpha_bf = sm.tile([BL, AN], BF16, tag="alphab", bufs=1)
            nc.vector.tensor_scalar_mul(alpha_bf[:, :], exp_sb[:, :], rcp[:, :])
            nc.sync.dma_start(out=alphas_p[:, t, :], in_=alpha_bf[:, :])

            # --- broadcast alpha -> SBUF bf16
            alb_sb = sc.tile([128, BL, AN], BF16, tag="bcast", bufs=1)
            for j in range(4):
                bc_ps = ps.tile([128, 2, AN], F32, tag=f"bc{j}", bufs=1)
                for i in range(2):
                    b = 2 * j + i
                    nc.tensor.matmul(
                        bc_ps[:, i, :],
                        oneh_sb[:, b * 128:(b + 1) * 128],
                        alpha_bf[:, :],
                        start=True,
                        stop=True,
                    )
                nc.scalar.activation(alb_sb[:, 2 * j:2 * j + 2, :], bc_ps[:, :, :], AF.Copy)

            # --- gates h-part slice B (hides z mul/reduce)
            gh_slice(2, 4, gh_ps, h_bf)

            # --- z = sum_n a*alb, then gates z-part (window-contiguous groups:
            #     start=True clears has_written for the whole bank, so a
            #     window's accumulation may not interleave with another's)

            z_f = sm.tile([128, OC * BL], F32, tag="zf", bufs=1)
            z_bf = sm.tile([128, OC * BL], BF16, tag="zbf", bufs=1)
            hs_sb = sm.tile([128, BL * (AN // 2)], BF16, tag="hsum", bufs=1)
            for vc in range(OC):
                nc.vector.tensor_tensor(
                    s_sb[:, vc, :], a_sb[:, vc, :],
                    alb_sb[:, :, :].rearrange("p b n -> p (b n)"),
                    op=OP.mult,
                )
                pv = s_sb[:, vc, :].rearrange("p (b h n) -> p b h n", b=BL, h=2)
                nc.vector.tensor_tensor(
                    hs_sb[:, :].rearrange("p (b n) -> p b n", b=BL),
                    pv[:, :, 0, :], pv[:, :, 1, :], op=OP.add,
                )
                nc.vector.reduce_sum(
                    z_f[:, vc * BL:(vc + 1) * BL],
                    hs_sb[:, :].rearrange("p (b n) -> p b n", b=BL),
                    axis=mybir.AxisListType.X,
                )
                nc.vector.tensor_copy(
                    z_bf[:, vc * BL:(vc + 1) * BL], z_f[:, vc * BL:(vc + 1) * BL])
            c_prev = c_f
            gall = sm.tile([128, 4, HC * BL], F32, tag="gall", bufs=1)
            sig = sm.tile([128, 3, HC * BL], F32, tag="sig", bufs=1)
            tg = sm.tile([128, HC * BL], F32, tag="tg", bufs=1)
            t1 = sm.tile([128, HC * BL], F32, tag="t1", bufs=1)
            t2 = sm.tile([128, HC * BL], F32, tag="t2", bufs=1)
            c_f = sm.tile([128, HC * BL], F32, tag="c")
            tc_f = sm.tile([128, HC * BL], F32, tag="tc", bufs=1)
            h_bf = sm.tile([128, HC * BL], BF16, tag="h")
            HB = HC * BL // 2
            # z-part accumulates straight onto gh_ps (bank pre-zeroed, so no
            # start=True anywhere -> groups need not be contiguous) k-OUTER:
            # each z chunk is consumed by 32 matmuls as soon as it reduces.
            for vc in range(OC):
                for g in range(4):
                    for hcj in range(HC):
                        m0 = g * H + hcj * 128
                        nc.tensor.matmul(
                            gh_ps[:, g, hcj * BL:(hcj + 1) * BL],
                            wcat_sb[:, vc, m0:m0 + 128],
                            z_bf[:, vc * BL:(vc + 1) * BL],
                            start=False,
                            stop=False,
                            skip_group_check=True,
                        )
            # --- LSTM cell (full width); gate order [i, f, o, g];
            #     sigmoid-free: th = tanh(x/2), sigmoid(x) = (1+th)/2,
            #     state kept as X = 2c.
            nc.vector.tensor_tensor(
                gall[:, :, :], gh_ps[:, :, :],
                gb_sb[:, :].rearrange("p (g c) -> p g c", g=4),
                op=OP.add,
            )
            nc.scalar.activation(sig[:, :, :], gall[:, 0:3, :], AF.Tanh,
                                 bias=0.0, scale=0.5)
            nc.scalar.activation(tg[:, :], gall[:, 3, :], AF.Tanh)
            nc.vector.scalar_tensor_tensor(
                t1[:, :], sig[:, 0, :], 1.0, tg[:, :],
                op0=OP.add, op1=OP.mult)                 # (1+th_i)·tanh(g)
            nc.vector.scalar_tensor_tensor(
                t2[:, :], sig[:, 1, :], 1.0, c_prev[:, :],
                op0=OP.add, op1=OP.mult)                 # (1+th_f)·X_prev
            nc.vector.scalar_tensor_tensor(
                c_f[:, :], t2[:, :], 0.5, t1[:, :],
                op0=OP.mult, op1=OP.add)                 # X = 2*c_new
            nc.scalar.activation(tc_f[:, :], c_f[:, :], AF.Tanh,
                                 bias=0.0, scale=0.5)    # tanh(c_new)
            nc.vector.tensor_scalar(
                sig[:, 0, :], sig[:, 2, :], 1.0, 0.5,
                op0=OP.add, op1=OP.mult)                 # sigmoid(o)
            nc.vector.tensor_mul(h_bf[:, :], sig[:, 0, :], tc_f[:, :])

            prev_h = h_bf

        emit_head(STEPS - 1, h_bf)
        nc.sync.dma_start(out=outs_p[:, :], in_=outs_sb[:, :])

    nc.compile()
    return nc


def _prep_inputs(inputs):
    ins = {k: np.asarray(v, np.float32) for k, v in inputs.items()}
    eps = 1e-5
    scale = ins["bn_gamma"] / np.sqrt(ins["bn_var"] + eps)
    wc = ins["inconv_w"] * scale[:, None]                     # [512, 2048]
    bc = ins["inconv_b"] * scale + ins["bn_beta"] - ins["bn_mean"] * scale

    def pack_cols(v, ncol):  # [ncol*128] -> [128, ncol]
        return _f32(v.reshape(ncol, 128).T)

    def reord(w):  # [4096, ...] pytorch i,f,g,o -> i,f,o,g
        return np.concatenate([w[:H], w[H:2 * H], w[3 * H:], w[2 * H:3 * H]], axis=0)

    bb = reord(ins["b_ih"] + ins["b_hh"])                     # [4096]
    # gb[p, g*HC*BL + c*BL + b] = bb[g*1024 + c*128 + p]
    gb = np.repeat(bb.reshape(4, HC, 128).transpose(2, 0, 1).reshape(128, 4 * HC),
                   BL, axis=1)

    oneh = np.zeros((BL, BL * 128), np.float32)
    for b in range(BL):
        oneh[b, b * 128:(b + 1) * 128] = 1.0

    shared = {
        "convT": _bf(wc.T),
        "cb": pack_cols(bc, OC),
        "e1T": _bf(ins["e1_w"].T),
        "hs1T": _bf(ins["hs1_w"].T),
        "hc1T": _bf(ins["hc1_w"].T),
        "hsb": pack_cols(ins["hs1_b"], HC),
        "hcb": pack_cols(ins["hc1_b"], HC),
        "eh1T": _bf(ins["eh1_w"].T),
        "eh1b": _bf(ins["eh1_b"][None, :]),
        "eh3": _bf(ins["eh3_w"].reshape(OC, 128).T),
        "wcatT": _bf(reord(np.concatenate([ins["w_ih"], ins["w_hh"]], axis=1)).T),
        "gb": _f32(gb),
        "reg1T": _bf(ins["reg1_w"].T),
        "r1b": pack_cols(ins["reg1_b"], RG // 128),
        "reg4": _bf(ins["reg4_w"].reshape(RG // 128, 128).T),
        "oneh": _bf(oneh),
    }

    x = ins["x"].reshape(B, D, AN)
    in_maps = []
    for c in range(NC):
        xs = x[c * BL:(c + 1) * BL]                 # [8, 2048, 196]
        xs = xs.transpose(1, 0, 2).reshape(D, NF)   # [2048, 1568]
        m = dict(shared)
        m["x"] = _bf(xs)
        in_maps.append(m)
    return in_maps, float(ins["reg4_b"])


def _install_trace_hook():
    """The agent image's antenv lacks axon_hooks; synthesize it so
    run_bass_kernel_spmd(trace=True) can reach NTFF profiling."""
    import types

    try:
        from antenv.axon_hooks import get_axon_ntff_profile_hook  # noqa: F401
        return
    except ImportError:
        pass
    import antenv

    mod = types.ModuleType("antenv.axon_hooks")
    _h = [None]
    mod.set_axon_ntff_profile_hook = lambda h: _h.__setitem__(0, h)
    mod.get_axon_ntff_profile_hook = lambda: _h[0]
    sys.modules["antenv.axon_hooks"] = mod
    antenv.axon_hooks = mod
    sys.path.insert(0, "/root/.axon_site")
    from trn_agent_boot.trn_boot import _ntff_profile_via_ctypes

    hook = _ntff_profile_via_ctypes("/opt/axon/libaxon_pjrt.so")
    if hook is not None:
        mod.set_axon_ntff_profile_hook(hook)


def kernel(**inputs):
    global LAST_RESULT
    if "nc" not in _CACHE:
        _CACHE["W0"] = float(
            np.asarray(inputs["eh3_w"], np.float32)
            .astype(ml_dtypes.bfloat16).astype(np.float32).sum())
        _CACHE["nc"] = build_nc(_CACHE["W0"])
    if bool(int(os.environ.get("KERNEL_TRACE", "0"))):
        _install_trace_hook()
    nc = _CACHE["nc"]
    in_maps, reg4_b = _prep_inputs(inputs)
    res = run_bass_kernel_spmd(
        nc, in_maps, core_ids=list(range(NC)),
        trace=bool(int(os.environ.get("KERNEL_TRACE", "0"))),
    )
    LAST_RESULT = res
    out_seq = np.zeros((B, STEPS), np.float32)
    alphas = np.zeros((B, STEPS, AN), np.float32)
    for c in range(NC):
        r = res.results[c]
        out_seq[c * BL:(c + 1) * BL] = r["outs"].reshape(STEPS, BL).T + reg4_b
        alphas[c * BL:(c + 1) * BL] = np.asarray(r["alphas"], np.float32)
    return out_seq, alphas
